# revision 1
# baseline (speedup 1.0000x reference)
"""Trainium2 Bass kernel for a quantized (FP4 e2m1, group-64 scales) MoE layer.

Problem shape (hardcoded): T=2048 tokens, K=2048 hidden, I=1024 intermediate,
E=8 routed experts (top-2), plus an always-on shared expert.

Strategy (8 NeuronCores):
  * Expert-parallel: core e owns routed expert e. The token->expert all-to-all
    is done host-side: for each expert we gather the tokens routed to it
    (merged top-2 slots, capacity C=512) and ship x^T [K, C] in bf16.
  * FP4 handling: the host unpacks the 4-bit fields to fp8_e4m3 (holding
    exactly 2*fp4_value - all exact in e4m3); the device applies the group
    scales (x0.5 folded in) with one tensor_tensor multiply per element
    (split across VectorE and GpSimdE) into SBUF-resident bf16 weights, then
    runs bf16 matmuls with fp32 PSUM accumulation.
  * Permuted contraction orderings: rows of the gate_up operands use
    k' = (c,p) -> k = (p%32)*64 + 4c + p//32 so that every 128-row chunk
    needs scale rows p%32 - one constant [128, N] scale tile serves all
    chunks (no 64x scale replication). Same idea for the down contraction:
    i' = 128c + p -> i = 8p + c, realized on the gate_up side by
    single-stride stationary-operand column APs (step 8, offset c), so
    activations emerge already i'-ordered and the down scale tile is also
    chunk-invariant (lane p -> scale row p//8).
  * Shared expert: token-split, 256 tokens per core; weights streamed through
    the same SBUF pools after the routed phases release them.
  * DMAs are batched into multi-chunk transfers (per-DMA fixed cost ~2us).
  * Combine (scatter-add by routing weights + shared add) on host.
"""

import numpy as np
import ml_dtypes

import concourse.bacc as bacc
import concourse.bass as bass
import concourse.mybir as mybir
import concourse.tile as tile
from concourse import bass_utils, library_config

F32 = mybir.dt.float32
BF16 = mybir.dt.bfloat16
FP8 = mybir.dt.float8e4

NP_BF16 = ml_dtypes.bfloat16
NP_FP8 = ml_dtypes.float8_e4m3

T, K, I, E, TOPK, GS = 2048, 2048, 1024, 8, 2, 64
N_CORES = 8
C = 512            # routed token capacity per expert (max merged load is 511
                   # for the fixed seed; host fallback handles any overflow)
TS = T // N_CORES  # shared-expert tokens per core = 256

KC = K // 128      # 16 contraction chunks for gate_up
IC = I // 128      # 8 contraction chunks for down
KS = K // 512      # 4 output column slices

# 2 * fp4_e2m1 value per nibble (sign bit 3): exact in fp8_e4m3 / bf16.
FP4_2T = np.array(
    [0, 1, 2, 3, 4, 6, 8, 12, 0, -1, -2, -3, -4, -6, -8, -12], dtype=np.float32
)

# Contraction permutations (see module docstring).
_kp = np.arange(K)
KPERM = (_kp % 128 % 32) * 64 + 4 * (_kp // 128) + (_kp % 128) // 32
_ip = np.arange(I)
IPERM = 8 * (_ip % 128) + (_ip // 128)

_GU_LANES = (np.arange(128) % 32)
_D_LANES = (np.arange(128) // 8)

_COMPILED = {}


def _decode_fp8_pairs(packed: np.ndarray, perm: np.ndarray) -> np.ndarray:
    """[R, N] int32 -> fp8 of 2*val, rows permuted, packed as chunk pairs
    [R*8//256, 128, 2N]."""
    shifts = (np.arange(8, dtype=np.int32) * 4)[None, :, None]
    nib = (packed[:, None, :] >> shifts) & 0xF
    vals = FP4_2T[nib].reshape(packed.shape[0] * 8, packed.shape[1])[perm]
    R, N = vals.shape
    out = vals.reshape(R // 256, 2, 128, N).transpose(0, 2, 1, 3)
    return np.ascontiguousarray(out.reshape(R // 256, 128, 2 * N)).astype(NP_FP8)


def _quad_chunks(mat: np.ndarray) -> np.ndarray:
    """[R, N] -> [R//512, 128, 4N] (4 row-chunks side by side)."""
    R, N = mat.shape
    out = mat.reshape(R // 512, 4, 128, N).transpose(0, 2, 1, 3)
    return np.ascontiguousarray(out.reshape(R // 512, 128, 4 * N))


def _scale128(scales: np.ndarray, lane_map: np.ndarray) -> np.ndarray:
    return (scales.astype(np.float32)[lane_map] * 0.5).astype(NP_BF16)


def _build_program(reps=1):
    """Build + compile the SPMD Bass program (identical on every core).
    reps>1 repeats the whole body (for timing-slope measurements)."""
    nc = bacc.Bacc("TRN2", target_bir_lowering=False, debug=False,
                   num_devices=N_CORES)

    # ---- DRAM I/O ----
    xT = nc.dram_tensor("xT", [KC // 4, 128, 4 * C], BF16, kind="ExternalInput")
    probs = nc.dram_tensor("probs", [128, C // 128], F32, kind="ExternalInput")
    v_gu = nc.dram_tensor("v_gu", [KC // 2, 128, 2 * 2 * I], FP8,
                          kind="ExternalInput")
    v_d = nc.dram_tensor("v_d", [IC // 2, 128, 2 * K], FP8,
                         kind="ExternalInput")
    s_gu = nc.dram_tensor("s_gu", [128, 2 * I], BF16, kind="ExternalInput")
    s_rest = nc.dram_tensor("s_rest", [128, 3 * 2048], BF16,
                            kind="ExternalInput")
    xsT = nc.dram_tensor("xsT", [KC // 4, 128, 4 * TS], BF16,
                         kind="ExternalInput")
    vs_gu = nc.dram_tensor("vs_gu", [KC // 2, 128, 2 * 2 * I], FP8,
                           kind="ExternalInput")
    vs_d = nc.dram_tensor("vs_d", [IC // 2, 128, 2 * K], FP8,
                          kind="ExternalInput")
    y = nc.dram_tensor("y", [C, K], F32, kind="ExternalOutput")
    ysh = nc.dram_tensor("ysh", [TS, K], F32, kind="ExternalOutput")

    with tile.TileContext(nc) as tc:
        with (
            tc.tile_pool(name="wgu", bufs=KC + 4) as wgu_pool,
            tc.tile_pool(name="wd", bufs=IC + 2) as wd_pool,
            tc.tile_pool(name="xt", bufs=KC // 4) as xt_pool,
            tc.tile_pool(name="xst", bufs=KC // 4) as xst_pool,
            tc.tile_pool(name="act", bufs=IC) as act_pool,
            tc.tile_pool(name="vq", bufs=3) as vq_pool,
            tc.tile_pool(name="vqp", bufs=3) as vqp_pool,
            tc.tile_pool(name="scl", bufs=1) as scl_pool,
            tc.tile_pool(name="ysb", bufs=2) as ysb_pool,
            tc.tile_pool(name="pr", bufs=1) as pr_pool,
            tc.tile_pool(name="silu", bufs=2) as silu_pool,
            tc.tile_pool(name="ps", bufs=8, space="PSUM") as psum_pool,
        ):
            # load the GPSIMD library up front - the auto-inserted reload
            # would otherwise be isolation-scheduled after DVE quiesces
            nc.gpsimd.load_library(library_config.standard)

            for _rep in range(reps):
                # ---- constant scale tiles (gate_up scales first: they gate the
                # first dequant; the rest is deferred below the hot loads) ----
                sgu_t = scl_pool.tile([128, 2 * I], BF16, tag="scl1")
                nc.scalar.dma_start(sgu_t[:, 0:I], s_gu[:, 0:I])
                nc.scalar.dma_start(sgu_t[:, I:2 * I], s_gu[:, I:2 * I])

                def chain_stages(stages):
                    # keep per-engine dequant queues in stage order; the
                    # scheduler otherwise reorders them by heap priority
                    last = {}
                    for tts in stages:
                        first_of, last_of = {}, {}
                        for eng, ti in tts:
                            first_of.setdefault(id(eng), ti)
                            last_of[id(eng)] = ti
                        for k, ti in first_of.items():
                            if k in last:
                                # ti depends on last[k] (runs after it)
                                tile.add_dep_helper(ti.ins, last[k].ins,
                                                    sync=False,
                                                    reason="dequant stage order")
                        last.update(last_of)

                def dequant_matrix(v_dram, npairs, scale_ap, pool, tag, ncols,
                                   engine_of, split_first=False, dma_order=None,
                                   pool_pairs=()):
                    vts = {}
                    tt_insts = []
                    for j in dma_order or range(npairs):
                        if j in pool_pairs:
                            vt = vqp_pool.tile([128, 2 * ncols], FP8, tag="vqp")
                        else:
                            vt = vq_pool.tile([128, 2 * ncols], FP8, tag="vq")
                        nsub = 4 if (split_first and j == 0) else 1
                        sub = 2 * ncols // nsub
                        for u in range(nsub):
                            nc.sync.dma_start(vt[:, u * sub:(u + 1) * sub],
                                              v_dram[j, :, u * sub:(u + 1) * sub])
                        vts[j] = vt
                    tiles = []
                    for ch in range(2 * npairs):
                        j, h = ch // 2, ch % 2
                        vt = vts[j]
                        wt = pool.tile([128, ncols], BF16, tag=tag)
                        eng = engine_of(ch)
                        if split_first and j == 0:  # halve the startup dep chain
                            for u in range(2):
                                ti = eng.tensor_tensor(
                                    wt[:, u * ncols // 2:(u + 1) * ncols // 2],
                                    vt[:, (2 * h + u) * ncols // 2:
                                          (2 * h + u + 1) * ncols // 2],
                                    scale_ap[:, u * ncols // 2:
                                             (u + 1) * ncols // 2],
                                    mybir.AluOpType.mult)
                        else:
                            ti = eng.tensor_tensor(
                                wt[:], vt[:, h * ncols:(h + 1) * ncols],
                                scale_ap, mybir.AluOpType.mult)
                        tiles.append(wt)
                        tt_insts.append((eng, ti))
                    return tiles, tt_insts

                def mlp(wgu_tiles, wd_tiles, xt_of, tcnt, y_dram, pr_ap):
                    """gate_up matmul + silu*up + down matmul + combine-scale."""
                    tchunks = tcnt // 128
                    # -- gate_up: for each down-chunk c, produce act'[c] [128, t]
                    # directly in i'-row order via strided stationary columns.
                    act_tiles = []
                    for c in range(IC):
                        hpair = []
                        for half in range(2):     # 0: gate, 1: up
                            ps = psum_pool.tile([128, tcnt], F32, tag="ps")
                            for k in range(KC):
                                lhs = (wgu_tiles[k][:, half * I:(half + 1) * I]
                                       .rearrange("p (r g) -> p g r",
                                                  r=128, g=8)[:, c, :])
                                nc.tensor.matmul(
                                    ps[:], lhs, xt_of(k),
                                    start=(k == 0), stop=(k == KC - 1),
                                )
                            hpair.append(ps)
                        gate_ps, up_ps = hpair
                        sil = silu_pool.tile([128, tcnt], BF16, tag="silu")
                        nc.scalar.activation(sil[:], gate_ps[:],
                                             mybir.ActivationFunctionType.Silu)
                        at = act_pool.tile([128, tcnt], BF16, tag="act")
                        nc.vector.tensor_tensor(at[:], sil[:], up_ps[:],
                                                mybir.AluOpType.mult)
                        act_tiles.append(at)

                    # -- down: y[t, k] = act'[i', t].T @ Wd'[i', k], x probs
                    for tb in range(tchunks):
                        last_tb = tb == tchunks - 1
                        for kh in range(2):
                            ot = ysb_pool.tile([128, K // 2], F32, tag="ysb")
                            for ks in (2 * kh, 2 * kh + 1):
                                ps = psum_pool.tile([128, 512], F32, tag="ps")
                                for c in range(IC):
                                    nc.tensor.matmul(
                                        ps[:],
                                        act_tiles[c][:, tb * 128:(tb + 1) * 128],
                                        wd_tiles[c][:, ks * 512:(ks + 1) * 512],
                                        start=(c == 0), stop=(c == IC - 1),
                                    )
                                osl = ot[:, (ks % 2) * 512:(ks % 2 + 1) * 512]
                                if pr_ap is None:
                                    if last_tb and ks >= KS - 2:
                                        # final copies split ACT/DVE, small
                                        # pieces -> short kernel tail
                                        for u in range(2):
                                            sl = osl[:, u * 256:(u + 1) * 256]
                                            pp = ps[:, u * 256:(u + 1) * 256]
                                            if u == 0:
                                                nc.scalar.copy(sl, pp)
                                            else:
                                                nc.vector.tensor_copy(sl, pp)
                                    else:
                                        nc.scalar.copy(osl, ps[:])
                                else:
                                    nc.scalar.activation(
                                        osl, ps[:],
                                        mybir.ActivationFunctionType.Copy,
                                        scale=pr_ap[:, tb:tb + 1])
                                if last_tb:   # shorten the kernel tail
                                    if pr_ap is None and ks == KS - 1:
                                        nc.sync.dma_start(
                                            y_dram[tb * 128:(tb + 1) * 128,
                                                   ks * 512:ks * 512 + 256],
                                            osl[:, 0:256])
                                        nc.scalar.dma_start(
                                            y_dram[tb * 128:(tb + 1) * 128,
                                                   ks * 512 + 256:(ks + 1) * 512],
                                            osl[:, 256:512])
                                    else:
                                        nc.sync.dma_start(
                                            y_dram[tb * 128:(tb + 1) * 128,
                                                   ks * 512:(ks + 1) * 512], osl)
                            if not last_tb:
                                nc.sync.dma_start(
                                    y_dram[tb * 128:(tb + 1) * 128,
                                           kh * 1024:(kh + 1) * 1024], ot[:])

                # ---- routed expert ----
                xt_tiles = []
                for q in range(KC // 4):
                    xt_t = xt_pool.tile([128, 4 * C], BF16, tag="xt")
                    nc.scalar.dma_start(xt_t[:], xT[q, :, :])
                    xt_tiles.append(xt_t)

                def xt_of(k):
                    return xt_tiles[k // 4][:, (k % 4) * C:(k % 4 + 1) * C]

                wgu_tiles, gu_tts = dequant_matrix(
                    v_gu, KC // 2, sgu_t[:], wgu_pool, "wgu", 2 * I,
                    lambda i: nc.vector if i < 10 else nc.gpsimd,
                    split_first=True, dma_order=[5, 0, 1, 2, 3, 6, 4, 7],
                    pool_pairs=(5, 6, 7))

                srest_t = scl_pool.tile([128, 3 * 2048], BF16, tag="scl2")
                nc.sync.dma_start(srest_t[:], s_rest[:, :])
                sd_t = srest_t[:, 0:2048]
                ssgu_t = srest_t[:, 2048:4096]
                ssd_t = srest_t[:, 4096:6144]
                pr_t = pr_pool.tile([128, C // 128], F32, tag="pr")
                nc.sync.dma_start(pr_t[:], probs[:, :])

                wd_tiles, wd_tts = dequant_matrix(
                    v_d, IC // 2, sd_t, wd_pool, "wd", K,
                    lambda i: nc.gpsimd if i < 4 else nc.vector,
                    pool_pairs=(0, 1))

                xst_tiles = []
                for q in range(KC // 4):
                    xs_t = xst_pool.tile([128, 4 * TS], BF16, tag="xst")
                    nc.sync.dma_start(xs_t[:], xsT[q, :, :])
                    xst_tiles.append(xs_t)

                def xst_of(k):
                    return xst_tiles[k // 4][:, (k % 4) * TS:(k % 4 + 1) * TS]

                mlp(wgu_tiles, wd_tiles, xt_of, C, y, pr_t)

                # ---- shared expert (reuses the weight pools' SBUF) ----

                wsgu_tiles, wsgu_tts = dequant_matrix(
                    vs_gu, KC // 2, ssgu_t, wgu_pool, "wgu", 2 * I,
                    lambda i: nc.vector if i < 10 else nc.gpsimd,
                    pool_pairs=(5, 6, 7))
                wsd_tiles, wsd_tts = dequant_matrix(
                    vs_d, IC // 2, ssd_t, wd_pool, "wd", K,
                    lambda i: nc.vector if i < 6 else nc.gpsimd,
                    pool_pairs=(3,))
                chain_stages([gu_tts, wd_tts, wsgu_tts, wsd_tts])

                mlp(wsgu_tiles, wsd_tiles, xst_of, TS, ysh, None)

    nc.compile()
    return nc


def _get_program():
    if "nc" not in _COMPILED:
        _COMPILED["nc"] = _build_program()
    return _COMPILED["nc"]


def kernel(**inputs) -> np.ndarray:
    x = np.asarray(inputs["hidden_states"], np.float32)          # [T, K]
    gu_p = np.asarray(inputs["gate_up_weight_packed"])           # [E, K/8, 2I]
    gu_s = np.asarray(inputs["gate_up_scales"], np.float32)      # [E, K/GS, 2I]
    d_p = np.asarray(inputs["down_weight_packed"])               # [E, I/8, K]
    d_s = np.asarray(inputs["down_scales"], np.float32)          # [E, I/GS, K]
    sgu_p = np.asarray(inputs["shared_gate_up_packed"])          # [K/8, 2I]
    sgu_s = np.asarray(inputs["shared_gate_up_scales"], np.float32)
    sd_p = np.asarray(inputs["shared_down_packed"])              # [I/8, K]
    sd_s = np.asarray(inputs["shared_down_scales"], np.float32)
    eids = np.asarray(inputs["expert_ids"])                      # [T, TOPK]
    eprobs = np.asarray(inputs["expert_probs"], np.float32)      # [T, TOPK]

    # ---- host routing: merged combine weights, token gather per expert ----
    combine = np.zeros((T, E), np.float32)
    np.add.at(combine, (np.arange(T)[:, None], eids), eprobs)
    idx_list = [np.nonzero(combine[:, e])[0] for e in range(E)]
    overflow = max(len(i) for i in idx_list) > C

    xbf = x.astype(NP_BF16)
    xbf_perm_T = np.ascontiguousarray(xbf.T[KPERM])              # [K, T]
    shared_vgu = _decode_fp8_pairs(sgu_p, KPERM)
    shared_vd = _decode_fp8_pairs(sd_p, IPERM)

    in_maps = []
    for e in range(E):
        idx = idx_list[e][:C]
        xT_e = np.zeros((K, C), NP_BF16)
        xT_e[:, :len(idx)] = xbf_perm_T[:, idx]
        pr_full = np.zeros(C, np.float32)
        pr_full[:len(idx)] = combine[idx, e]
        pr_e = np.ascontiguousarray(pr_full.reshape(C // 128, 128).T)
        s_rest_e = np.concatenate(
            [_scale128(d_s[e], _D_LANES),
             _scale128(sgu_s, _GU_LANES),
             _scale128(sd_s, _D_LANES)], axis=1)
        in_maps.append({
            "xT": _quad_chunks(xT_e),
            "probs": pr_e,
            "v_gu": _decode_fp8_pairs(gu_p[e], KPERM),
            "s_gu": _scale128(gu_s[e], _GU_LANES),
            "v_d": _decode_fp8_pairs(d_p[e], IPERM),
            "s_rest": np.ascontiguousarray(s_rest_e),
            "xsT": _quad_chunks(
                np.ascontiguousarray(xbf_perm_T[:, e * TS:(e + 1) * TS])),
            "vs_gu": shared_vgu,
            "vs_d": shared_vd,
        })

    nc = _get_program()
    res = bass_utils.run_bass_kernel_spmd(nc, in_maps,
                                          core_ids=list(range(N_CORES)))

    # ---- host combine ----
    out = np.zeros((T, K), np.float32)
    for e in range(E):
        idx = idx_list[e][:C]
        out[idx] += res.results[e]["y"][:len(idx)]
        out[e * TS:(e + 1) * TS] += res.results[e]["ysh"]

    if overflow:
        # pathological load imbalance: finish dropped tokens on host (exact)
        for e in range(E):
            extra = idx_list[e][C:]
            if len(extra) == 0:
                continue
            wgu = _dequant_full(gu_p[e], gu_s[e])
            wd = _dequant_full(d_p[e], d_s[e])
            h = x[extra] @ wgu
            g, u = h[:, :I], h[:, I:]
            a = (g / (1 + np.exp(-g))) * u
            out[extra] += (a @ wd) * combine[extra, e][:, None]
    return out


def _dequant_full(packed, scales):
    shifts = (np.arange(8, dtype=np.int32) * 4)[None, :, None]
    nib = (packed[:, None, :] >> shifts) & 0xF
    w = FP4_2T[nib].reshape(packed.shape[0] * 8, packed.shape[1]) * 0.5
    return w * np.repeat(scales.astype(np.float32), GS, axis=0)



# revision 10
# speedup vs baseline: 1.1573x; 1.1573x over previous
"""Trainium2 Bass kernel for a quantized (FP4 e2m1, group-64 scales) MoE layer.

FP8 DoubleRow edition: all matmuls run as fp8e4 (IEEE e4m3, max 240)
DoubleRow matmuls (2 k-chunks per instruction). The host pre-scales and
pre-quantizes everything; the device does zero dequantization.

Numerics (validated against the reference on the fixed seed, rel ~1.3e-2):
  * gate weights: fp8(16*Wg) + fp8 residual (shipped, extra matmul pass)
  * up weights:   fp8(4*Wu) routed / fp8(2*Wu) shared (plain)
  * down weights: fp8(16*Wd) + fp8 residual
  * activations x: fp8(x) + fp8 residual (two moving passes)
  * act = silu(g)*u: computed in bf16, re-quantized to fp8 + fp8 residual
  * outputs fp16, combine probs folded into the ACT-engine copy scale.

Sharding: expert-parallel (core e owns routed expert e, capacity C=512)
plus a 256-token slice of the always-on shared expert per core. Token
gather/scatter and combine run on host.
"""

import numpy as np
import ml_dtypes

import concourse.bacc as bacc
import concourse.bass as bass
import concourse.mybir as mybir
import concourse.tile as tile
from concourse import bass_utils, library_config

F32 = mybir.dt.float32
BF16 = mybir.dt.bfloat16
F16 = mybir.dt.float16
FP8 = mybir.dt.float8e4
DR = mybir.MatmulPerfMode.DoubleRow
Copy = mybir.ActivationFunctionType.Copy
Silu = mybir.ActivationFunctionType.Silu
Mult = mybir.AluOpType.mult
Sub = mybir.AluOpType.subtract

NP_BF16 = ml_dtypes.bfloat16
NP_F8 = ml_dtypes.float8_e4m3          # IEEE e4m3: max 240, min normal 2^-7

T, K, I, E, TOPK, GS = 2048, 2048, 1024, 8, 2, 64
N_CORES = 8
C = 512            # routed token capacity per expert
TS = T // N_CORES  # shared-expert tokens per core = 256
KP = K // 256      # 8 contraction chunk-pairs for gate_up
IP = I // 256      # 4 contraction chunk-pairs for down

FP4_T = np.array([0, .5, 1, 1.5, 2, 3, 4, 6,
                  0, -.5, -1, -1.5, -2, -3, -4, -6], dtype=np.float32)

_COMPILED = {}


# ---------------------------------------------------------------- host prep
def _decode(packed, scales):
    """[R, N] int32 + [R*8//GS, N] scales -> [R*8, N] f32 true weights."""
    shifts = (np.arange(8, dtype=np.int32)[None, :, None] * 4)
    nib = (packed[:, None, :] >> shifts) & 0xF
    w = FP4_T[nib].reshape(packed.shape[0] * 8, packed.shape[1])
    return w * np.repeat(scales.astype(np.float32), GS, axis=0)


def _pairs(mat, block):
    """[R, N] -> [R//(256*block), 128, block*2N]: chunk pairs, `block` pairs
    side by side per DMA-able row block."""
    R, N = mat.shape
    p = mat.reshape(R // 256, 2, 128, N).transpose(0, 2, 1, 3)
    p = p.reshape(R // 256, 128, 2 * N)
    g = p.reshape(R // 256 // block, block, 128, 2 * N).transpose(0, 2, 1, 3)
    return np.ascontiguousarray(g.reshape(R // 256 // block, 128, block * 2 * N))


def _f8(a):
    return np.asarray(a, np.float32).astype(NP_F8)


def _quant_gu(wtrue, up_scale):
    """-> (w8 packed [4,128,8192], wl_gate packed [2,128,8192])."""
    wg = 16.0 * wtrue[:, :I]
    wu = up_scale * wtrue[:, I:]
    w8 = _f8(np.concatenate([wg, wu], axis=1))
    wl = _f8(wg - w8[:, :I].astype(np.float32))
    return _pairs(w8, 2), _pairs(wl, 4)


def _quant_d(wtrue):
    w16 = 16.0 * wtrue
    w8 = _f8(w16)
    wl = _f8(w16 - w8.astype(np.float32))
    return _pairs(w8, 2), _pairs(wl, 2)


# ---------------------------------------------------------------- device
def _build_program(reps=1):
    nc = bacc.Bacc("TRN2", target_bir_lowering=False, debug=False,
                   num_devices=N_CORES)

    xh_d = nc.dram_tensor("xh", [2, 128, 4096], FP8, kind="ExternalInput")
    xl_d = nc.dram_tensor("xl", [2, 128, 4096], FP8, kind="ExternalInput")
    xsh_d = nc.dram_tensor("xsh", [128, 4096], FP8, kind="ExternalInput")
    xsl_d = nc.dram_tensor("xsl", [128, 4096], FP8, kind="ExternalInput")
    wgu_d = nc.dram_tensor("wgu", [4, 128, 8192], FP8, kind="ExternalInput")
    wgl_d = nc.dram_tensor("wgl", [2, 128, 8192], FP8, kind="ExternalInput")
    wd_d = nc.dram_tensor("wd", [2, 128, 8192], FP8, kind="ExternalInput")
    wdl_d = nc.dram_tensor("wdl", [2, 128, 8192], FP8, kind="ExternalInput")
    swgu_d = nc.dram_tensor("swgu", [4, 128, 8192], FP8, kind="ExternalInput")
    swgl_d = nc.dram_tensor("swgl", [2, 128, 8192], FP8, kind="ExternalInput")
    swd_d = nc.dram_tensor("swd", [2, 128, 8192], FP8, kind="ExternalInput")
    swdl_d = nc.dram_tensor("swdl", [2, 128, 8192], FP8, kind="ExternalInput")
    pr_d = nc.dram_tensor("pr", [128, C // 128], F32, kind="ExternalInput")
    y_d = nc.dram_tensor("y", [C, K], F16, kind="ExternalOutput")
    ysh_d = nc.dram_tensor("ysh", [TS, K], F16, kind="ExternalOutput")

    def two(ap):
        return ap.rearrange("p (two n) -> p two n", two=2)

    with tile.TileContext(nc) as tc:
        with (
            tc.tile_pool(name="wgu", bufs=8) as wgu_pool,
            tc.tile_pool(name="wgl", bufs=3) as wgl_pool,
            tc.tile_pool(name="wd", bufs=3) as wd_pool,
            tc.tile_pool(name="wdl", bufs=3) as wdl_pool,
            tc.tile_pool(name="x", bufs=7) as x_pool,
            tc.tile_pool(name="a8", bufs=6) as a8_pool,
            tc.tile_pool(name="al8", bufs=6) as al8_pool,
            tc.tile_pool(name="sil", bufs=4) as sil_pool,
            tc.tile_pool(name="abf", bufs=4) as abf_pool,
            tc.tile_pool(name="ysb", bufs=18) as ysb_pool,
            tc.tile_pool(name="pr", bufs=1) as pr_pool,
            tc.tile_pool(name="ps", bufs=8, space="PSUM") as psum_pool,
        ):
            nc.gpsimd.load_library(library_config.standard)

            for _rep in range(reps):
                # ---------- input DMA stream (sync queue, priority order)
                def load(pool, dram, idx, cols, tag):
                    t = pool.tile([128, cols], FP8, tag=tag)
                    nc.sync.dma_start(t[:], dram[idx, :, :] if idx is not None
                                      else dram[:, :])
                    return t

                # first-needed pieces at pair granularity so PE starts early
                xh0 = x_pool.tile([128, 4096], FP8, tag="x", name="xh0")
                nc.sync.dma_start(xh0[:, 0:1024], xh_d[0, :, 0:1024])
                wgu0 = wgu_pool.tile([128, 8192], FP8, tag="wgu", name="wgu0")
                nc.sync.dma_start(wgu0[:, 0:4096], wgu_d[0, :, 0:4096])
                xl0 = x_pool.tile([128, 4096], FP8, tag="x", name="xl0")
                nc.sync.dma_start(xl0[:, 0:1024], xl_d[0, :, 0:1024])
                nc.sync.dma_start(xh0[:, 1024:4096], xh_d[0, :, 1024:4096])
                nc.sync.dma_start(xl0[:, 1024:4096], xl_d[0, :, 1024:4096])
                nc.sync.dma_start(wgu0[:, 4096:8192], wgu_d[0, :, 4096:8192])
                xh_t = [xh0]
                xl_t = [xl0]
                wgu_t = [wgu0, load(wgu_pool, wgu_d, 1, 8192, "wgu")]
                xh_t.append(load(x_pool, xh_d, 1, 4096, "x"))
                xl_t.append(load(x_pool, xl_d, 1, 4096, "x"))
                wgu_t += [load(wgu_pool, wgu_d, 2, 8192, "wgu"),
                          load(wgu_pool, wgu_d, 3, 8192, "wgu")]
                wgl_t = [load(wgl_pool, wgl_d, q, 8192, "wgl") for q in range(2)]
                wd_t = [load(wd_pool, wd_d, q, 8192, "wd") for q in range(2)]
                wdl_t = [load(wdl_pool, wdl_d, q, 8192, "wdl") for q in range(2)]
                pr_t = pr_pool.tile([128, C // 128], F32, tag="pr")
                nc.sync.dma_start(pr_t[:], pr_d[:, :])
                xsh_t = [load(x_pool, xsh_d, None, 4096, "x")]
                xsl_t = [load(x_pool, xsl_d, None, 4096, "x")]
                swgu_t = [load(wgu_pool, swgu_d, q, 8192, "wgu")
                          for q in range(4)]
                swgl_t = [load(wgl_pool, swgl_d, q, 8192, "wgl")
                          for q in range(2)]
                swd_t = [load(wd_pool, swd_d, q, 8192, "wd") for q in range(2)]
                swdl_t = [load(wdl_pool, swdl_d, q, 8192, "wdl")
                          for q in range(2)]

                # AP helpers ------------------------------------------------
                def wgu_ap(tiles, j, w0, w1):
                    # pair j of gate_up weights, col window [w0, w1)
                    t = tiles[j // 2]
                    return two(t[:, (j % 2) * 4096:(j % 2 + 1) * 4096])[:, :, w0:w1]

                def wgl_ap(tiles, j, w0, w1):
                    t = tiles[j // 4]
                    return two(t[:, (j % 4) * 2048:(j % 4 + 1) * 2048])[:, :, w0:w1]

                def x_ap(tiles, j, tcnt):
                    if tcnt == C:
                        t = tiles[j // 4]
                        return two(t[:, (j % 4) * 1024:(j % 4 + 1) * 1024])
                    return two(tiles[0][:, j * 512:(j + 1) * 512])

                def wd_ap(tiles, p, w0, w1):
                    t = tiles[p // 2]
                    return two(t[:, (p % 2) * 4096:(p % 2 + 1) * 4096])[:, :, w0:w1]

                def mlp(wgu_tl, wgl_tl, wd_tl, wdl_tl, xh_tl, xl_tl, tcnt,
                        y_dram, pr_ap, actl):
                    """One expert pass: gate_up -> act -> down -> out."""
                    a8_tiles, al8_tiles = [], []
                    # ---- gate_up in waves of 2 chunks (4 psums) so PSUM
                    # banks hand off smoothly at phase boundaries ----
                    for wave in range(4):
                        cs = range(2 * wave, 2 * wave + 2)
                        gps = {c: psum_pool.tile([128, tcnt], F32, tag="ps",
                                                 name=f"gps{c}")
                               for c in cs}
                        ups = {c: psum_pool.tile([128, tcnt], F32, tag="ps",
                                                 name=f"ups{c}")
                               for c in cs}
                        for j in range(KP):
                            for c in cs:
                                nc.tensor.matmul(
                                    gps[c][:],
                                    wgu_ap(wgu_tl, j, c * 128, (c + 1) * 128),
                                    x_ap(xh_tl, j, tcnt),
                                    start=(j == 0), stop=False, perf_mode=DR)
                            for c in cs:
                                nc.tensor.matmul(
                                    ups[c][:],
                                    wgu_ap(wgu_tl, j, I + c * 128,
                                           I + (c + 1) * 128),
                                    x_ap(xh_tl, j, tcnt),
                                    start=(j == 0), stop=False, perf_mode=DR)
                            for c in cs:
                                nc.tensor.matmul(
                                    gps[c][:],
                                    wgu_ap(wgu_tl, j, c * 128, (c + 1) * 128),
                                    x_ap(xl_tl, j, tcnt),
                                    start=False, stop=False, perf_mode=DR)
                            for c in cs:
                                nc.tensor.matmul(
                                    ups[c][:],
                                    wgu_ap(wgu_tl, j, I + c * 128,
                                           I + (c + 1) * 128),
                                    x_ap(xl_tl, j, tcnt),
                                    start=False, stop=(j == KP - 1),
                                    perf_mode=DR)
                        # gate residual pass
                        for j in range(KP):
                            for c in cs:
                                nc.tensor.matmul(
                                    gps[c][:],
                                    wgl_ap(wgl_tl, j, c * 128, (c + 1) * 128),
                                    x_ap(xh_tl, j, tcnt),
                                    start=False, stop=(j == KP - 1),
                                    perf_mode=DR)
                        # act: sil = silu(g), a_bf = sil*up, a8 (+ al8) fp8
                        for c in cs:
                            cp, half = c // 2, c % 2
                            if half == 0:
                                a8_tiles.append(
                                    a8_pool.tile([128, 2 * tcnt], FP8, tag="a8",
                                                 name=f"a8_{cp}"))
                                if actl:
                                    al8_tiles.append(
                                        al8_pool.tile([128, 2 * tcnt], FP8,
                                                      tag="al8",
                                                      name=f"al8_{cp}"))
                            sil = sil_pool.tile([128, tcnt], BF16, tag="sil")
                            nc.scalar.activation(sil[:], gps[c][:], Silu,
                                                 scale=1.0 / 16)
                            abf = abf_pool.tile([128, tcnt], BF16, tag="abf")
                            nc.vector.tensor_tensor(abf[:], sil[:], ups[c][:],
                                                    Mult)
                            a8s = a8_tiles[cp][:, half * tcnt:(half + 1) * tcnt]
                            nc.gpsimd.tensor_copy(a8s, abf[:])
                            if actl:
                                nc.vector.tensor_tensor(
                                    al8_tiles[cp][:, half * tcnt:
                                                   (half + 1) * tcnt],
                                    abf[:], a8s, Sub)

                    # ---- down ----
                    tblocks = tcnt // 128
                    for tb in range(tblocks):
                        for kw in range(4):
                            ps = psum_pool.tile([128, 512], F32, tag="ps")
                            for cp in range(IP):
                                nc.tensor.matmul(
                                    ps[:],
                                    two(a8_tiles[cp][:])[:, :,
                                                         tb * 128:(tb + 1) * 128],
                                    wd_ap(wd_tl, cp, kw * 512, (kw + 1) * 512),
                                    start=(cp == 0), stop=False, perf_mode=DR)
                            if actl:
                                for cp in range(IP):
                                    nc.tensor.matmul(
                                        ps[:],
                                        two(al8_tiles[cp][:])[
                                            :, :, tb * 128:(tb + 1) * 128],
                                        wd_ap(wd_tl, cp, kw * 512,
                                              (kw + 1) * 512),
                                        start=False, stop=False, perf_mode=DR)
                            for cp in range(IP):
                                nc.tensor.matmul(
                                    ps[:],
                                    two(a8_tiles[cp][:])[:, :,
                                                         tb * 128:(tb + 1) * 128],
                                    wd_ap(wdl_tl, cp, kw * 512, (kw + 1) * 512),
                                    start=False, stop=(cp == IP - 1),
                                    perf_mode=DR)
                            ot = ysb_pool.tile([128, 512], F16, tag="ysb")
                            yslice = y_dram[tb * 128:(tb + 1) * 128,
                                            kw * 512:(kw + 1) * 512]
                            scale = (pr_ap[:, tb:tb + 1] if pr_ap is not None
                                     else 1.0 / 32)
                            last = tb == tblocks - 1 and kw == 3
                            if last and pr_ap is None:
                                # final tile: halve across ACT/DVE + two DMA
                                # queues to shorten the kernel tail
                                nc.scalar.activation(ot[:, 0:256],
                                                     ps[:, 0:256], Copy,
                                                     scale=scale)
                                nc.vector.tensor_scalar_mul(ot[:, 256:512],
                                                            ps[:, 256:512],
                                                            scale)
                                nc.scalar.dma_start(yslice[:, 0:256],
                                                    ot[:, 0:256])
                                nc.sync.dma_start(yslice[:, 256:512],
                                                  ot[:, 256:512])
                            else:
                                # alternate copy engines so PSUM banks free
                                # fast; all out-DMAs ride the SP queue (inputs
                                # are long since issued; ACT queue must stay
                                # clear of transfer-holding DMACopies)
                                if (tb * 4 + kw) % 2 == 0:
                                    nc.vector.tensor_scalar_mul(ot[:], ps[:],
                                                                scale)
                                else:
                                    nc.scalar.activation(ot[:], ps[:], Copy,
                                                         scale=scale)
                                nc.sync.dma_start(yslice, ot[:])

                mlp(wgu_t, wgl_t, wd_t, wdl_t, xh_t, xl_t, C, y_d, pr_t,
                    actl=False)
                mlp(swgu_t, swgl_t, swd_t, swdl_t, xsh_t, xsl_t, TS, ysh_d,
                    None, actl=True)

    nc.compile()
    return nc


def _get_program():
    if "nc" not in _COMPILED:
        _COMPILED["nc"] = _build_program()
    return _COMPILED["nc"]


# ---------------------------------------------------------------- entry
def kernel(**inputs) -> np.ndarray:
    x = np.asarray(inputs["hidden_states"], np.float32)
    gu_p = np.asarray(inputs["gate_up_weight_packed"])
    gu_s = np.asarray(inputs["gate_up_scales"], np.float32)
    d_p = np.asarray(inputs["down_weight_packed"])
    d_s = np.asarray(inputs["down_scales"], np.float32)
    sgu_p = np.asarray(inputs["shared_gate_up_packed"])
    sgu_s = np.asarray(inputs["shared_gate_up_scales"], np.float32)
    sd_p = np.asarray(inputs["shared_down_packed"])
    sd_s = np.asarray(inputs["shared_down_scales"], np.float32)
    eids = np.asarray(inputs["expert_ids"])
    eprobs = np.asarray(inputs["expert_probs"], np.float32)

    # host routing
    combine = np.zeros((T, E), np.float32)
    np.add.at(combine, (np.arange(T)[:, None], eids), eprobs)
    idx_list = [np.nonzero(combine[:, e])[0] for e in range(E)]
    overflow = max(len(i) for i in idx_list) > C

    # x quantization (hi + residual), transposed [K, T]
    xh8 = x.astype(NP_F8)
    xl8 = (x - xh8.astype(np.float32)).astype(NP_F8)
    xh8T = np.ascontiguousarray(xh8.T)
    xl8T = np.ascontiguousarray(xl8.T)

    swgu, swgl = _quant_gu(_decode(sgu_p, sgu_s), 2.0)
    swd, swdl = _quant_d(_decode(sd_p, sd_s))
    xsh_full = _pairs(xh8T, 8)[0]
    xsl_full = _pairs(xl8T, 8)[0]

    in_maps = []
    for e in range(E):
        idx = idx_list[e][:C]
        xh_e = np.zeros((K, C), NP_F8)
        xh_e[:, :len(idx)] = xh8T[:, idx]
        xl_e = np.zeros((K, C), NP_F8)
        xl_e[:, :len(idx)] = xl8T[:, idx]
        pr_full = np.zeros(C, np.float32)
        pr_full[:len(idx)] = combine[idx, e] / 64.0
        wgu, wgl = _quant_gu(_decode(gu_p[e], gu_s[e]), 4.0)
        wd, wdl = _quant_d(_decode(d_p[e], d_s[e]))
        in_maps.append({
            "xh": _pairs(xh_e, 4),
            "xl": _pairs(xl_e, 4),
            "xsh": _pairs(np.ascontiguousarray(xh8T[:, e * TS:(e + 1) * TS]), 8)[0],
            "xsl": _pairs(np.ascontiguousarray(xl8T[:, e * TS:(e + 1) * TS]), 8)[0],
            "wgu": wgu, "wgl": wgl, "wd": wd, "wdl": wdl,
            "swgu": swgu, "swgl": swgl, "swd": swd, "swdl": swdl,
            "pr": np.ascontiguousarray(pr_full.reshape(C // 128, 128).T),
        })

    nc = _get_program()
    res = bass_utils.run_bass_kernel_spmd(nc, in_maps,
                                          core_ids=list(range(N_CORES)))

    out = np.zeros((T, K), np.float32)
    for e in range(E):
        idx = idx_list[e][:C]
        out[idx] += res.results[e]["y"][:len(idx)].astype(np.float32)
        out[e * TS:(e + 1) * TS] += res.results[e]["ysh"].astype(np.float32)

    if overflow:
        for e in range(E):
            extra = idx_list[e][C:]
            if len(extra) == 0:
                continue
            wgu = _decode(gu_p[e], gu_s[e])
            wd = _decode(d_p[e], d_s[e])
            h = x[extra] @ wgu
            g, u = h[:, :I], h[:, I:]
            a = (g / (1 + np.exp(-g))) * u
            out[extra] += (a @ wd) * combine[extra, e][:, None]
    return out


# revision 18
# speedup vs baseline: 1.2088x; 1.0445x over previous
"""Trainium2 Bass kernel for a quantized (FP4 e2m1, group-64 scales) MoE layer.

FP8 DoubleRow edition: all matmuls run as fp8e4 (IEEE e4m3, max 240)
DoubleRow matmuls (2 k-chunks per instruction). The host pre-scales and
pre-quantizes everything; the device does zero dequantization.

Numerics (validated against the reference on the fixed seed, rel ~1.3e-2):
  * gate weights: fp8(16*Wg) + fp8 residual (shipped, extra matmul pass)
  * up weights:   fp8(4*Wu) routed / fp8(2*Wu) shared (plain)
  * down weights: fp8(16*Wd) + fp8 residual
  * activations x: fp8(x) + fp8 residual (two moving passes)
  * act = silu(g)*u: computed in bf16, re-quantized to fp8 + fp8 residual
  * outputs fp16, combine probs folded into the ACT-engine copy scale.

Sharding: expert-parallel (core e owns routed expert e, capacity C=512)
plus a 256-token slice of the always-on shared expert per core. Token
gather/scatter and combine run on host.
"""

import numpy as np
import ml_dtypes

import concourse.bacc as bacc
import concourse.bass as bass
import concourse.mybir as mybir
import concourse.tile as tile
from concourse import bass_utils, library_config

F32 = mybir.dt.float32
BF16 = mybir.dt.bfloat16
F16 = mybir.dt.float16
FP8 = mybir.dt.float8e4
DR = mybir.MatmulPerfMode.DoubleRow
Copy = mybir.ActivationFunctionType.Copy
Silu = mybir.ActivationFunctionType.Silu
Mult = mybir.AluOpType.mult
Sub = mybir.AluOpType.subtract

NP_BF16 = ml_dtypes.bfloat16
NP_F8 = ml_dtypes.float8_e4m3          # IEEE e4m3: max 240, min normal 2^-7

T, K, I, E, TOPK, GS = 2048, 2048, 1024, 8, 2, 64
N_CORES = 8
C = 512            # routed token capacity per expert
TS = T // N_CORES  # shared-expert tokens per core = 256
KP = K // 256      # 8 contraction chunk-pairs for gate_up
IP = I // 256      # 4 contraction chunk-pairs for down

FP4_T = np.array([0, .5, 1, 1.5, 2, 3, 4, 6,
                  0, -.5, -1, -1.5, -2, -3, -4, -6], dtype=np.float32)

_COMPILED = {}


# ---------------------------------------------------------------- host prep
def _decode(packed, scales):
    """[R, N] int32 + [R*8//GS, N] scales -> [R*8, N] f32 true weights."""
    shifts = (np.arange(8, dtype=np.int32)[None, :, None] * 4)
    nib = (packed[:, None, :] >> shifts) & 0xF
    w = FP4_T[nib].reshape(packed.shape[0] * 8, packed.shape[1])
    return w * np.repeat(scales.astype(np.float32), GS, axis=0)


def _pairs(mat, block):
    """[R, N] -> [R//(256*block), 128, block*2N]: chunk pairs, `block` pairs
    side by side per DMA-able row block."""
    R, N = mat.shape
    p = mat.reshape(R // 256, 2, 128, N).transpose(0, 2, 1, 3)
    p = p.reshape(R // 256, 128, 2 * N)
    g = p.reshape(R // 256 // block, block, 128, 2 * N).transpose(0, 2, 1, 3)
    return np.ascontiguousarray(g.reshape(R // 256 // block, 128, block * 2 * N))


def _f8(a):
    return np.asarray(a, np.float32).astype(NP_F8)


def _quant_gu(wtrue, up_scale):
    """-> (w8 wave-blocks [4,128,8192], wl_gate wave-blocks [4,128,4096]).

    Wave w (output chunks 2w, 2w+1) owns gate cols [256w:256w+256) and up
    cols [I+256w:...). Each wave block packs those 512 columns for all 16
    contraction chunks so a wave's weights arrive in one ~1MB stream."""
    wg = 16.0 * wtrue[:, :I]
    wu = up_scale * wtrue[:, I:]
    w8 = _f8(np.concatenate([wg, wu], axis=1))
    wl = _f8(wg - w8[:, :I].astype(np.float32))
    wgu_w = np.stack([_pairs(np.concatenate(
        [w8[:, 256 * w:256 * w + 256], w8[:, I + 256 * w:I + 256 * w + 256]],
        axis=1), 8)[0] for w in range(4)])
    wgl_w = np.stack([_pairs(wl[:, 256 * w:256 * w + 256], 8)[0]
                      for w in range(4)])
    return wgu_w, wgl_w


def _quant_d(wtrue):
    w16 = 16.0 * wtrue
    w8 = _f8(w16)
    wl = _f8(w16 - w8.astype(np.float32))
    return _pairs(w8, 2), _pairs(wl, 2)


# ---------------------------------------------------------------- device
def _build_program(reps=1):
    nc = bacc.Bacc("TRN2", target_bir_lowering=False, debug=False,
                   num_devices=N_CORES)

    xh_d = nc.dram_tensor("xh", [2, 128, 4096], FP8, kind="ExternalInput")
    xl_d = nc.dram_tensor("xl", [2, 128, 4096], FP8, kind="ExternalInput")
    xsh_d = nc.dram_tensor("xsh", [128, 4096], FP8, kind="ExternalInput")
    xsl_d = nc.dram_tensor("xsl", [128, 4096], FP8, kind="ExternalInput")
    wgu_d = nc.dram_tensor("wgu", [4, 128, 8192], FP8, kind="ExternalInput")
    wgl_d = nc.dram_tensor("wgl", [4, 128, 4096], FP8, kind="ExternalInput")
    wd_d = nc.dram_tensor("wd", [2, 128, 8192], FP8, kind="ExternalInput")
    wdl_d = nc.dram_tensor("wdl", [2, 128, 8192], FP8, kind="ExternalInput")
    swgu_d = nc.dram_tensor("swgu", [4, 128, 8192], FP8, kind="ExternalInput")
    swgl_d = nc.dram_tensor("swgl", [4, 128, 4096], FP8, kind="ExternalInput")
    swd_d = nc.dram_tensor("swd", [2, 128, 8192], FP8, kind="ExternalInput")
    swdl_d = nc.dram_tensor("swdl", [2, 128, 8192], FP8, kind="ExternalInput")
    pr_d = nc.dram_tensor("pr", [128, C // 128], F32, kind="ExternalInput")
    y_d = nc.dram_tensor("y", [C, K], F16, kind="ExternalOutput")
    ysh_d = nc.dram_tensor("ysh", [TS, K], F16, kind="ExternalOutput")

    def two(ap):
        return ap.rearrange("p (two n) -> p two n", two=2)

    with tile.TileContext(nc) as tc:
        with (
            tc.tile_pool(name="wgu", bufs=8) as wgu_pool,
            tc.tile_pool(name="wgl", bufs=6) as wgl_pool,
            tc.tile_pool(name="wd", bufs=3) as wd_pool,
            tc.tile_pool(name="wdl", bufs=3) as wdl_pool,
            tc.tile_pool(name="x", bufs=7) as x_pool,
            tc.tile_pool(name="a8", bufs=6) as a8_pool,
            tc.tile_pool(name="al8", bufs=6) as al8_pool,
            tc.tile_pool(name="sil", bufs=4) as sil_pool,
            tc.tile_pool(name="abf", bufs=4) as abf_pool,
            tc.tile_pool(name="ysb", bufs=18) as ysb_pool,
            tc.tile_pool(name="pr", bufs=1) as pr_pool,
            tc.tile_pool(name="ps", bufs=8, space="PSUM") as psum_pool,
        ):
            nc.gpsimd.load_library(library_config.standard)

            for _rep in range(reps):
                # ---------- input DMA stream (sync queue, priority order)
                def load(pool, dram, idx, cols, tag):
                    t = pool.tile([128, cols], FP8, tag=tag)
                    nc.sync.dma_start(t[:], dram[idx, :, :] if idx is not None
                                      else dram[:, :])
                    return t

                # first-needed pieces at pair granularity so PE starts early
                xh0 = x_pool.tile([128, 4096], FP8, tag="x", name="xh0")
                nc.sync.dma_start(xh0[:, 0:1024], xh_d[0, :, 0:1024])
                wgu0 = wgu_pool.tile([128, 8192], FP8, tag="wgu", name="wgu0")
                nc.sync.dma_start(wgu0[:, 0:4096], wgu_d[0, :, 0:4096])
                xl0 = x_pool.tile([128, 4096], FP8, tag="x", name="xl0")
                nc.sync.dma_start(xl0[:, 0:1024], xl_d[0, :, 0:1024])
                nc.sync.dma_start(xh0[:, 1024:4096], xh_d[0, :, 1024:4096])
                nc.sync.dma_start(xl0[:, 1024:4096], xl_d[0, :, 1024:4096])
                nc.sync.dma_start(wgu0[:, 4096:8192], wgu_d[0, :, 4096:8192])
                xh_t = [xh0]
                xl_t = [xl0]
                wgu_t = [wgu0]
                xh_t.append(load(x_pool, xh_d, 1, 4096, "x"))
                xl_t.append(load(x_pool, xl_d, 1, 4096, "x"))
                wgl_t = [load(wgl_pool, wgl_d, 0, 4096, "wgl")]
                for q in range(1, 4):
                    wgu_t.append(load(wgu_pool, wgu_d, q, 8192, "wgu"))
                    wgl_t.append(load(wgl_pool, wgl_d, q, 4096, "wgl"))
                wd_t = [load(wd_pool, wd_d, q, 8192, "wd") for q in range(2)]
                wdl_t = [load(wdl_pool, wdl_d, q, 8192, "wdl") for q in range(2)]
                pr_t = pr_pool.tile([128, C // 128], F32, tag="pr")
                nc.sync.dma_start(pr_t[:], pr_d[:, :])
                xsh_t = [load(x_pool, xsh_d, None, 4096, "x")]
                xsl_t = [load(x_pool, xsl_d, None, 4096, "x")]
                swgu_t, swgl_t = [], []
                for q in range(4):
                    swgu_t.append(load(wgu_pool, swgu_d, q, 8192, "wgu"))
                    swgl_t.append(load(wgl_pool, swgl_d, q, 4096, "wgl"))
                swd_t = [load(wd_pool, swd_d, q, 8192, "wd") for q in range(2)]
                swdl_t = [load(wdl_pool, swdl_d, q, 8192, "wdl")
                          for q in range(2)]

                # AP helpers ------------------------------------------------
                def wgu_ap(tiles, w, j, i0):
                    # wave w, pair j of gate_up weights, 128-col window at i0
                    # (0/128 = gate c, 256/384 = up c within the wave block)
                    return two(tiles[w][:, j * 1024:(j + 1) * 1024])[
                        :, :, i0:i0 + 128]

                def wgl_ap(tiles, w, j, cw):
                    return two(tiles[w][:, j * 512:(j + 1) * 512])[
                        :, :, cw * 128:(cw + 1) * 128]

                def x_ap(tiles, j, tcnt):
                    if tcnt == C:
                        t = tiles[j // 4]
                        return two(t[:, (j % 4) * 1024:(j % 4 + 1) * 1024])
                    return two(tiles[0][:, j * 512:(j + 1) * 512])

                def wd_ap(tiles, p, w0, w1):
                    t = tiles[p // 2]
                    return two(t[:, (p % 2) * 4096:(p % 2 + 1) * 4096])[:, :, w0:w1]

                def mlp(wgu_tl, wgl_tl, wd_tl, wdl_tl, xh_tl, xl_tl, tcnt,
                        y_dram, pr_ap, actl):
                    """One expert pass: gate_up -> act -> down -> out."""
                    a8_tiles, al8_tiles = [], []
                    # ---- gate_up in waves of 2 chunks (4 psums) so PSUM
                    # banks hand off smoothly at phase boundaries ----
                    for wave in range(4):
                        cs = range(2 * wave, 2 * wave + 2)
                        gps = {c: psum_pool.tile([128, tcnt], F32, tag="ps",
                                                 name=f"gps{c}")
                               for c in cs}
                        ups = {c: psum_pool.tile([128, tcnt], F32, tag="ps",
                                                 name=f"ups{c}")
                               for c in cs}
                        for j in range(KP):
                            for c in cs:
                                nc.tensor.matmul(
                                    gps[c][:],
                                    wgu_ap(wgu_tl, wave, j, (c % 2) * 128),
                                    x_ap(xh_tl, j, tcnt),
                                    start=(j == 0), stop=False, perf_mode=DR)
                            for c in cs:
                                nc.tensor.matmul(
                                    ups[c][:],
                                    wgu_ap(wgu_tl, wave, j,
                                           256 + (c % 2) * 128),
                                    x_ap(xh_tl, j, tcnt),
                                    start=(j == 0), stop=False, perf_mode=DR)
                            for c in cs:
                                nc.tensor.matmul(
                                    gps[c][:],
                                    wgu_ap(wgu_tl, wave, j, (c % 2) * 128),
                                    x_ap(xl_tl, j, tcnt),
                                    start=False, stop=False, perf_mode=DR)
                            for c in cs:
                                nc.tensor.matmul(
                                    ups[c][:],
                                    wgu_ap(wgu_tl, wave, j,
                                           256 + (c % 2) * 128),
                                    x_ap(xl_tl, j, tcnt),
                                    start=False, stop=(j == KP - 1),
                                    perf_mode=DR)
                        # gate residual pass
                        for j in range(KP):
                            for c in cs:
                                nc.tensor.matmul(
                                    gps[c][:],
                                    wgl_ap(wgl_tl, wave, j, c % 2),
                                    x_ap(xh_tl, j, tcnt),
                                    start=False, stop=(j == KP - 1),
                                    perf_mode=DR)
                        # act: sil = silu(g), a_bf = sil*up, a8 (+ al8) fp8
                        for c in cs:
                            cp, half = c // 2, c % 2
                            if half == 0:
                                a8_tiles.append(
                                    a8_pool.tile([128, 2 * tcnt], FP8, tag="a8",
                                                 name=f"a8_{cp}"))
                                if actl:
                                    al8_tiles.append(
                                        al8_pool.tile([128, 2 * tcnt], FP8,
                                                      tag="al8",
                                                      name=f"al8_{cp}"))
                            sil = sil_pool.tile([128, tcnt], BF16, tag="sil")
                            nc.scalar.activation(sil[:], gps[c][:], Silu,
                                                 scale=1.0 / 16)
                            abf = abf_pool.tile([128, tcnt], BF16, tag="abf")
                            nc.vector.tensor_tensor(abf[:], sil[:], ups[c][:],
                                                    Mult)
                            a8s = a8_tiles[cp][:, half * tcnt:(half + 1) * tcnt]
                            nc.gpsimd.tensor_copy(a8s, abf[:])
                            if actl:
                                nc.vector.tensor_tensor(
                                    al8_tiles[cp][:, half * tcnt:
                                                   (half + 1) * tcnt],
                                    abf[:], a8s, Sub)

                    # ---- down ----
                    tblocks = tcnt // 128
                    for tb in range(tblocks):
                        for kw in range(4):
                            ps = psum_pool.tile([128, 512], F32, tag="ps")
                            for cp in range(IP):
                                nc.tensor.matmul(
                                    ps[:],
                                    two(a8_tiles[cp][:])[:, :,
                                                         tb * 128:(tb + 1) * 128],
                                    wd_ap(wd_tl, cp, kw * 512, (kw + 1) * 512),
                                    start=(cp == 0), stop=False, perf_mode=DR)
                            if actl:
                                for cp in range(IP):
                                    nc.tensor.matmul(
                                        ps[:],
                                        two(al8_tiles[cp][:])[
                                            :, :, tb * 128:(tb + 1) * 128],
                                        wd_ap(wd_tl, cp, kw * 512,
                                              (kw + 1) * 512),
                                        start=False, stop=False, perf_mode=DR)
                            for cp in range(IP):
                                nc.tensor.matmul(
                                    ps[:],
                                    two(a8_tiles[cp][:])[:, :,
                                                         tb * 128:(tb + 1) * 128],
                                    wd_ap(wdl_tl, cp, kw * 512, (kw + 1) * 512),
                                    start=False, stop=(cp == IP - 1),
                                    perf_mode=DR)
                            ot = ysb_pool.tile([128, 512], F16, tag="ysb")
                            yslice = y_dram[tb * 128:(tb + 1) * 128,
                                            kw * 512:(kw + 1) * 512]
                            scale = (pr_ap[:, tb:tb + 1] if pr_ap is not None
                                     else 1.0 / 32)
                            last = tb == tblocks - 1 and kw == 3
                            if last and pr_ap is None:
                                # final tile: halve across ACT/DVE + two DMA
                                # queues to shorten the kernel tail
                                nc.scalar.activation(ot[:, 0:256],
                                                     ps[:, 0:256], Copy,
                                                     scale=scale)
                                nc.vector.tensor_scalar_mul(ot[:, 256:512],
                                                            ps[:, 256:512],
                                                            scale)
                                nc.scalar.dma_start(yslice[:, 0:256],
                                                    ot[:, 0:256])
                                nc.sync.dma_start(yslice[:, 256:512],
                                                  ot[:, 256:512])
                            else:
                                # alternate copy engines so PSUM banks free
                                # fast; all out-DMAs ride the SP queue (inputs
                                # are long since issued; ACT queue must stay
                                # clear of transfer-holding DMACopies)
                                if (tb * 4 + kw) % 2 == 0:
                                    nc.vector.tensor_scalar_mul(ot[:], ps[:],
                                                                scale)
                                else:
                                    nc.scalar.activation(ot[:], ps[:], Copy,
                                                         scale=scale)
                                nc.sync.dma_start(yslice, ot[:])

                mlp(wgu_t, wgl_t, wd_t, wdl_t, xh_t, xl_t, C, y_d, pr_t,
                    actl=False)
                mlp(swgu_t, swgl_t, swd_t, swdl_t, xsh_t, xsl_t, TS, ysh_d,
                    None, actl=True)

    nc.compile()
    return nc


def _get_program():
    if "nc" not in _COMPILED:
        _COMPILED["nc"] = _build_program()
    return _COMPILED["nc"]


# ---------------------------------------------------------------- entry
def kernel(**inputs) -> np.ndarray:
    x = np.asarray(inputs["hidden_states"], np.float32)
    gu_p = np.asarray(inputs["gate_up_weight_packed"])
    gu_s = np.asarray(inputs["gate_up_scales"], np.float32)
    d_p = np.asarray(inputs["down_weight_packed"])
    d_s = np.asarray(inputs["down_scales"], np.float32)
    sgu_p = np.asarray(inputs["shared_gate_up_packed"])
    sgu_s = np.asarray(inputs["shared_gate_up_scales"], np.float32)
    sd_p = np.asarray(inputs["shared_down_packed"])
    sd_s = np.asarray(inputs["shared_down_scales"], np.float32)
    eids = np.asarray(inputs["expert_ids"])
    eprobs = np.asarray(inputs["expert_probs"], np.float32)

    # host routing
    combine = np.zeros((T, E), np.float32)
    np.add.at(combine, (np.arange(T)[:, None], eids), eprobs)
    idx_list = [np.nonzero(combine[:, e])[0] for e in range(E)]
    overflow = max(len(i) for i in idx_list) > C

    # x quantization (hi + residual), transposed [K, T]
    xh8 = x.astype(NP_F8)
    xl8 = (x - xh8.astype(np.float32)).astype(NP_F8)
    xh8T = np.ascontiguousarray(xh8.T)
    xl8T = np.ascontiguousarray(xl8.T)

    swgu, swgl = _quant_gu(_decode(sgu_p, sgu_s), 2.0)
    swd, swdl = _quant_d(_decode(sd_p, sd_s))
    xsh_full = _pairs(xh8T, 8)[0]
    xsl_full = _pairs(xl8T, 8)[0]

    in_maps = []
    for e in range(E):
        idx = idx_list[e][:C]
        xh_e = np.zeros((K, C), NP_F8)
        xh_e[:, :len(idx)] = xh8T[:, idx]
        xl_e = np.zeros((K, C), NP_F8)
        xl_e[:, :len(idx)] = xl8T[:, idx]
        pr_full = np.zeros(C, np.float32)
        pr_full[:len(idx)] = combine[idx, e] / 64.0
        wgu, wgl = _quant_gu(_decode(gu_p[e], gu_s[e]), 4.0)
        wd, wdl = _quant_d(_decode(d_p[e], d_s[e]))
        in_maps.append({
            "xh": _pairs(xh_e, 4),
            "xl": _pairs(xl_e, 4),
            "xsh": _pairs(np.ascontiguousarray(xh8T[:, e * TS:(e + 1) * TS]), 8)[0],
            "xsl": _pairs(np.ascontiguousarray(xl8T[:, e * TS:(e + 1) * TS]), 8)[0],
            "wgu": wgu, "wgl": wgl, "wd": wd, "wdl": wdl,
            "swgu": swgu, "swgl": swgl, "swd": swd, "swdl": swdl,
            "pr": np.ascontiguousarray(pr_full.reshape(C // 128, 128).T),
        })

    nc = _get_program()
    res = bass_utils.run_bass_kernel_spmd(nc, in_maps,
                                          core_ids=list(range(N_CORES)))

    out = np.zeros((T, K), np.float32)
    for e in range(E):
        idx = idx_list[e][:C]
        out[idx] += res.results[e]["y"][:len(idx)].astype(np.float32)
        out[e * TS:(e + 1) * TS] += res.results[e]["ysh"].astype(np.float32)

    if overflow:
        for e in range(E):
            extra = idx_list[e][C:]
            if len(extra) == 0:
                continue
            wgu = _decode(gu_p[e], gu_s[e])
            wd = _decode(d_p[e], d_s[e])
            h = x[extra] @ wgu
            g, u = h[:, :I], h[:, I:]
            a = (g / (1 + np.exp(-g))) * u
            out[extra] += (a @ wd) * combine[extra, e][:, None]
    return out


# revision 20
# speedup vs baseline: 1.2356x; 1.0222x over previous
"""Trainium2 Bass kernel for a quantized (FP4 e2m1, group-64 scales) MoE layer.

FP8 DoubleRow edition: all matmuls run as fp8e4 (IEEE e4m3, max 240)
DoubleRow matmuls (2 k-chunks per instruction). The host pre-scales and
pre-quantizes everything; the device does zero dequantization.

Numerics (validated against the reference on the fixed seed, rel ~1.3e-2):
  * gate weights: fp8(16*Wg) + fp8 residual (shipped, extra matmul pass)
  * up weights:   fp8(4*Wu) routed / fp8(2*Wu) shared (plain)
  * down weights: fp8(16*Wd) + fp8 residual
  * activations x: fp8(x) + fp8 residual (two moving passes)
  * act = silu(g)*u: computed in bf16, re-quantized to fp8 + fp8 residual
  * outputs fp16, combine probs folded into the ACT-engine copy scale.

Sharding: expert-parallel (core e owns routed expert e, capacity C=512)
plus a 256-token slice of the always-on shared expert per core. Token
gather/scatter and combine run on host.
"""

import numpy as np
import ml_dtypes

import concourse.bacc as bacc
import concourse.bass as bass
import concourse.mybir as mybir
import concourse.tile as tile
from concourse import bass_utils, library_config

F32 = mybir.dt.float32
BF16 = mybir.dt.bfloat16
F16 = mybir.dt.float16
FP8 = mybir.dt.float8e4
DR = mybir.MatmulPerfMode.DoubleRow
Copy = mybir.ActivationFunctionType.Copy
Silu = mybir.ActivationFunctionType.Silu
Mult = mybir.AluOpType.mult
Sub = mybir.AluOpType.subtract

NP_BF16 = ml_dtypes.bfloat16
NP_F8 = ml_dtypes.float8_e4m3          # IEEE e4m3: max 240, min normal 2^-7

T, K, I, E, TOPK, GS = 2048, 2048, 1024, 8, 2, 64
N_CORES = 8
C = 512            # routed token capacity per expert
TS = T // N_CORES  # shared-expert tokens per core = 256
KP = K // 256      # 8 contraction chunk-pairs for gate_up
IP = I // 256      # 4 contraction chunk-pairs for down

FP4_T = np.array([0, .5, 1, 1.5, 2, 3, 4, 6,
                  0, -.5, -1, -1.5, -2, -3, -4, -6], dtype=np.float32)

_COMPILED = {}


# ---------------------------------------------------------------- host prep
def _decode(packed, scales):
    """[R, N] int32 + [R*8//GS, N] scales -> [R*8, N] f32 true weights."""
    shifts = (np.arange(8, dtype=np.int32)[None, :, None] * 4)
    nib = (packed[:, None, :] >> shifts) & 0xF
    w = FP4_T[nib].reshape(packed.shape[0] * 8, packed.shape[1])
    return w * np.repeat(scales.astype(np.float32), GS, axis=0)


def _pairs(mat, block):
    """[R, N] -> [R//(256*block), 128, block*2N]: chunk pairs, `block` pairs
    side by side per DMA-able row block."""
    R, N = mat.shape
    p = mat.reshape(R // 256, 2, 128, N).transpose(0, 2, 1, 3)
    p = p.reshape(R // 256, 128, 2 * N)
    g = p.reshape(R // 256 // block, block, 128, 2 * N).transpose(0, 2, 1, 3)
    return np.ascontiguousarray(g.reshape(R // 256 // block, 128, block * 2 * N))


def _f8(a):
    return np.asarray(a, np.float32).astype(NP_F8)


def _quant_gu(wtrue, up_scale):
    """-> (w8 wave-blocks [4,128,8192], wl_gate wave-blocks [4,128,4096]).

    Wave w (output chunks 2w, 2w+1) owns gate cols [256w:256w+256) and up
    cols [I+256w:...). Each wave block packs those 512 columns for all 16
    contraction chunks so a wave's weights arrive in one ~1MB stream."""
    wg = 16.0 * wtrue[:, :I]
    wu = up_scale * wtrue[:, I:]
    w8 = _f8(np.concatenate([wg, wu], axis=1))
    wl = _f8(wg - w8[:, :I].astype(np.float32))
    wgu_w = np.stack([_pairs(np.concatenate(
        [w8[:, 256 * w:256 * w + 256], w8[:, I + 256 * w:I + 256 * w + 256]],
        axis=1), 8)[0] for w in range(4)])
    wgl_w = np.stack([_pairs(wl[:, 256 * w:256 * w + 256], 8)[0]
                      for w in range(4)])
    return wgu_w, wgl_w


def _quant_d(wtrue):
    w16 = 16.0 * wtrue
    w8 = _f8(w16)
    wl = _f8(w16 - w8.astype(np.float32))
    return _pairs(w8, 2), _pairs(wl, 2)


# ---------------------------------------------------------------- device
def _build_program(reps=1):
    nc = bacc.Bacc("TRN2", target_bir_lowering=False, debug=False,
                   num_devices=N_CORES)

    xh_d = nc.dram_tensor("xh", [2, 128, 4096], FP8, kind="ExternalInput")
    xl_d = nc.dram_tensor("xl", [2, 128, 4096], FP8, kind="ExternalInput")
    xsh_d = nc.dram_tensor("xsh", [128, 4096], FP8, kind="ExternalInput")
    xsl_d = nc.dram_tensor("xsl", [128, 4096], FP8, kind="ExternalInput")
    wgu_d = nc.dram_tensor("wgu", [4, 128, 8192], FP8, kind="ExternalInput")
    wgl_d = nc.dram_tensor("wgl", [4, 128, 4096], FP8, kind="ExternalInput")
    wd_d = nc.dram_tensor("wd", [2, 128, 8192], FP8, kind="ExternalInput")
    wdl_d = nc.dram_tensor("wdl", [2, 128, 8192], FP8, kind="ExternalInput")
    swgu_d = nc.dram_tensor("swgu", [4, 128, 8192], FP8, kind="ExternalInput")
    swgl_d = nc.dram_tensor("swgl", [4, 128, 4096], FP8, kind="ExternalInput")
    swd_d = nc.dram_tensor("swd", [2, 128, 8192], FP8, kind="ExternalInput")
    swdl_d = nc.dram_tensor("swdl", [2, 128, 8192], FP8, kind="ExternalInput")
    pr_d = nc.dram_tensor("pr", [128, C // 128], F32, kind="ExternalInput")
    y_d = nc.dram_tensor("y", [C, K], F16, kind="ExternalOutput")
    ysh_d = nc.dram_tensor("ysh", [TS, K], F16, kind="ExternalOutput")

    def two(ap):
        return ap.rearrange("p (two n) -> p two n", two=2)

    with tile.TileContext(nc) as tc:
        with (
            tc.tile_pool(name="wgu", bufs=8) as wgu_pool,
            tc.tile_pool(name="wgl", bufs=6) as wgl_pool,
            tc.tile_pool(name="wd", bufs=3) as wd_pool,
            tc.tile_pool(name="wdl", bufs=3) as wdl_pool,
            tc.tile_pool(name="x", bufs=7) as x_pool,
            tc.tile_pool(name="a8", bufs=6) as a8_pool,
            tc.tile_pool(name="al8", bufs=6) as al8_pool,
            tc.tile_pool(name="sil", bufs=4) as sil_pool,
            tc.tile_pool(name="abf", bufs=4) as abf_pool,
            tc.tile_pool(name="ysb", bufs=18) as ysb_pool,
            tc.tile_pool(name="pr", bufs=1) as pr_pool,
            tc.tile_pool(name="ps", bufs=8, space="PSUM") as psum_pool,
        ):
            nc.gpsimd.load_library(library_config.standard)

            for _rep in range(reps):
                # ---------- input DMA stream (sync queue, priority order)
                def load(pool, dram, idx, cols, tag):
                    t = pool.tile([128, cols], FP8, tag=tag)
                    nc.sync.dma_start(t[:], dram[idx, :, :] if idx is not None
                                      else dram[:, :])
                    return t

                # first-needed pieces at pair granularity so PE starts early
                xh0 = x_pool.tile([128, 4096], FP8, tag="x", name="xh0")
                nc.sync.dma_start(xh0[:, 0:1024], xh_d[0, :, 0:1024])
                wgu0 = wgu_pool.tile([128, 8192], FP8, tag="wgu", name="wgu0")
                nc.sync.dma_start(wgu0[:, 0:4096], wgu_d[0, :, 0:4096])
                xl0 = x_pool.tile([128, 4096], FP8, tag="x", name="xl0")
                nc.sync.dma_start(xl0[:, 0:1024], xl_d[0, :, 0:1024])
                nc.sync.dma_start(xh0[:, 1024:4096], xh_d[0, :, 1024:4096])
                nc.sync.dma_start(xl0[:, 1024:4096], xl_d[0, :, 1024:4096])
                nc.sync.dma_start(wgu0[:, 4096:8192], wgu_d[0, :, 4096:8192])
                xh_t = [xh0]
                xl_t = [xl0]
                wgu_t = [wgu0]
                xh_t.append(load(x_pool, xh_d, 1, 4096, "x"))
                xl_t.append(load(x_pool, xl_d, 1, 4096, "x"))
                wgl_t = [load(wgl_pool, wgl_d, 0, 4096, "wgl")]
                for q in range(1, 4):
                    wgu_t.append(load(wgu_pool, wgu_d, q, 8192, "wgu"))
                    wgl_t.append(load(wgl_pool, wgl_d, q, 4096, "wgl"))
                wd_t = [load(wd_pool, wd_d, q, 8192, "wd") for q in range(2)]
                wdl_t = [load(wdl_pool, wdl_d, q, 8192, "wdl") for q in range(2)]
                pr_t = pr_pool.tile([128, C // 128], F32, tag="pr")
                nc.sync.dma_start(pr_t[:], pr_d[:, :])
                xsh_t = [load(x_pool, xsh_d, None, 4096, "x")]
                xsl_t = [load(x_pool, xsl_d, None, 4096, "x")]
                swgu_t, swgl_t = [], []
                for q in range(4):
                    swgu_t.append(load(wgu_pool, swgu_d, q, 8192, "wgu"))
                    swgl_t.append(load(wgl_pool, swgl_d, q, 4096, "wgl"))
                swd_t = [load(wd_pool, swd_d, q, 8192, "wd") for q in range(2)]
                swdl_t = [load(wdl_pool, swdl_d, q, 8192, "wdl")
                          for q in range(2)]

                # AP helpers ------------------------------------------------
                def wgu_ap(tiles, w, j, i0):
                    # wave w, pair j of gate_up weights, 128-col window at i0
                    # (0/128 = gate c, 256/384 = up c within the wave block)
                    return two(tiles[w][:, j * 1024:(j + 1) * 1024])[
                        :, :, i0:i0 + 128]

                def wgl_ap(tiles, w, j, cw):
                    return two(tiles[w][:, j * 512:(j + 1) * 512])[
                        :, :, cw * 128:(cw + 1) * 128]

                def x_ap(tiles, j, tcnt):
                    if tcnt == C:
                        t = tiles[j // 4]
                        return two(t[:, (j % 4) * 1024:(j % 4 + 1) * 1024])
                    return two(tiles[0][:, j * 512:(j + 1) * 512])

                def wd_ap(tiles, p, w0, w1):
                    t = tiles[p // 2]
                    return two(t[:, (p % 2) * 4096:(p % 2 + 1) * 4096])[:, :, w0:w1]

                def mlp(wgu_tl, wgl_tl, wd_tl, wdl_tl, xh_tl, xl_tl, tcnt,
                        y_dram, pr_ap, actl):
                    """One expert pass: gate_up -> act -> down -> out."""
                    a8_tiles, al8_tiles = [], []
                    # ---- gate_up in waves of 2 chunks (4 psums) so PSUM
                    # banks hand off smoothly at phase boundaries ----
                    for wave in range(4):
                        cs = range(2 * wave, 2 * wave + 2)
                        gps = {c: psum_pool.tile([128, tcnt], F32, tag="ps",
                                                 name=f"gps{c}")
                               for c in cs}
                        ups = {c: psum_pool.tile([128, tcnt], F32, tag="ps",
                                                 name=f"ups{c}")
                               for c in cs}
                        for j in range(KP):
                            for c in cs:
                                nc.tensor.matmul(
                                    gps[c][:],
                                    wgu_ap(wgu_tl, wave, j, (c % 2) * 128),
                                    x_ap(xh_tl, j, tcnt),
                                    start=(j == 0), stop=False, perf_mode=DR)
                            for c in cs:
                                nc.tensor.matmul(
                                    ups[c][:],
                                    wgu_ap(wgu_tl, wave, j,
                                           256 + (c % 2) * 128),
                                    x_ap(xh_tl, j, tcnt),
                                    start=(j == 0), stop=False, perf_mode=DR)
                            for c in cs:
                                nc.tensor.matmul(
                                    gps[c][:],
                                    wgu_ap(wgu_tl, wave, j, (c % 2) * 128),
                                    x_ap(xl_tl, j, tcnt),
                                    start=False, stop=False, perf_mode=DR)
                            for c in cs:
                                nc.tensor.matmul(
                                    ups[c][:],
                                    wgu_ap(wgu_tl, wave, j,
                                           256 + (c % 2) * 128),
                                    x_ap(xl_tl, j, tcnt),
                                    start=False, stop=(j == KP - 1),
                                    perf_mode=DR)
                        # gate residual pass
                        for j in range(KP):
                            for c in cs:
                                nc.tensor.matmul(
                                    gps[c][:],
                                    wgl_ap(wgl_tl, wave, j, c % 2),
                                    x_ap(xh_tl, j, tcnt),
                                    start=False, stop=(j == KP - 1),
                                    perf_mode=DR)
                        # act: sil = silu(g), a_bf = sil*up, a8 (+ al8) fp8
                        for c in cs:
                            cp, half = c // 2, c % 2
                            if half == 0:
                                a8_tiles.append(
                                    a8_pool.tile([128, 2 * tcnt], FP8, tag="a8",
                                                 name=f"a8_{cp}"))
                                if actl:
                                    al8_tiles.append(
                                        al8_pool.tile([128, 2 * tcnt], FP8,
                                                      tag="al8",
                                                      name=f"al8_{cp}"))
                            sil = sil_pool.tile([128, tcnt], BF16, tag="sil")
                            nc.scalar.activation(sil[:], gps[c][:], Silu,
                                                 scale=1.0 / 16)
                            abf = abf_pool.tile([128, tcnt], BF16, tag="abf")
                            nc.vector.tensor_tensor(abf[:], sil[:], ups[c][:],
                                                    Mult)
                            a8s = a8_tiles[cp][:, half * tcnt:(half + 1) * tcnt]
                            nc.gpsimd.tensor_copy(a8s, abf[:])
                            if actl:
                                nc.vector.tensor_tensor(
                                    al8_tiles[cp][:, half * tcnt:
                                                   (half + 1) * tcnt],
                                    abf[:], a8s, Sub)

                    # ---- down ----
                    tblocks = tcnt // 128
                    for tb in range(tblocks):
                        for kw in range(4):
                            ps = psum_pool.tile([128, 512], F32, tag="ps")
                            for cp in range(IP):
                                nc.tensor.matmul(
                                    ps[:],
                                    two(a8_tiles[cp][:])[:, :,
                                                         tb * 128:(tb + 1) * 128],
                                    wd_ap(wd_tl, cp, kw * 512, (kw + 1) * 512),
                                    start=(cp == 0), stop=False, perf_mode=DR)
                            if actl:
                                for cp in range(IP):
                                    nc.tensor.matmul(
                                        ps[:],
                                        two(al8_tiles[cp][:])[
                                            :, :, tb * 128:(tb + 1) * 128],
                                        wd_ap(wd_tl, cp, kw * 512,
                                              (kw + 1) * 512),
                                        start=False, stop=False, perf_mode=DR)
                            for cp in range(IP):
                                nc.tensor.matmul(
                                    ps[:],
                                    two(a8_tiles[cp][:])[:, :,
                                                         tb * 128:(tb + 1) * 128],
                                    wd_ap(wdl_tl, cp, kw * 512, (kw + 1) * 512),
                                    start=False, stop=(cp == IP - 1),
                                    perf_mode=DR)
                            ot = ysb_pool.tile([128, 512], F16, tag="ysb")
                            yslice = y_dram[tb * 128:(tb + 1) * 128,
                                            kw * 512:(kw + 1) * 512]
                            scale = (pr_ap[:, tb:tb + 1] if pr_ap is not None
                                     else 1.0 / 32)
                            last = tb == tblocks - 1 and kw == 3
                            if last and pr_ap is None:
                                # final tile: halve across ACT/DVE + two DMA
                                # queues to shorten the kernel tail
                                nc.scalar.activation(ot[:, 0:256],
                                                     ps[:, 0:256], Copy,
                                                     scale=scale)
                                nc.vector.tensor_scalar_mul(ot[:, 256:512],
                                                            ps[:, 256:512],
                                                            scale)
                                nc.scalar.dma_start(yslice[:, 0:256],
                                                    ot[:, 0:256])
                                nc.sync.dma_start(yslice[:, 256:512],
                                                  ot[:, 256:512])
                            else:
                                # alternate copy engines so PSUM banks free
                                # fast; all out-DMAs ride the SP queue (inputs
                                # are long since issued; ACT queue must stay
                                # clear of transfer-holding DMACopies)
                                if (tb * 4 + kw) % 2 == 0:
                                    nc.vector.tensor_scalar_mul(ot[:], ps[:],
                                                                scale)
                                else:
                                    nc.scalar.activation(ot[:], ps[:], Copy,
                                                         scale=scale)
                                nc.sync.dma_start(yslice, ot[:])

                mlp(wgu_t, wgl_t, wd_t, wdl_t, xh_t, xl_t, C, y_d, pr_t,
                    actl=False)
                mlp(swgu_t, swgl_t, swd_t, swdl_t, xsh_t, xsl_t, TS, ysh_d,
                    None, actl=True)

    nc.compile()
    return nc


def _get_program():
    if "nc" not in _COMPILED:
        _COMPILED["nc"] = _build_program()
    return _COMPILED["nc"]


# ---------------------------------------------------------------- entry
def kernel(**inputs) -> np.ndarray:
    x = np.asarray(inputs["hidden_states"], np.float32)
    gu_p = np.asarray(inputs["gate_up_weight_packed"])
    gu_s = np.asarray(inputs["gate_up_scales"], np.float32)
    d_p = np.asarray(inputs["down_weight_packed"])
    d_s = np.asarray(inputs["down_scales"], np.float32)
    sgu_p = np.asarray(inputs["shared_gate_up_packed"])
    sgu_s = np.asarray(inputs["shared_gate_up_scales"], np.float32)
    sd_p = np.asarray(inputs["shared_down_packed"])
    sd_s = np.asarray(inputs["shared_down_scales"], np.float32)
    eids = np.asarray(inputs["expert_ids"])
    eprobs = np.asarray(inputs["expert_probs"], np.float32)

    # host routing
    combine = np.zeros((T, E), np.float32)
    np.add.at(combine, (np.arange(T)[:, None], eids), eprobs)
    idx_list = [np.nonzero(combine[:, e])[0] for e in range(E)]
    overflow = max(len(i) for i in idx_list) > C

    # x quantization (hi + residual), transposed [K, T]
    xh8 = x.astype(NP_F8)
    xl8 = (x - xh8.astype(np.float32)).astype(NP_F8)
    xh8T = np.ascontiguousarray(xh8.T)
    xl8T = np.ascontiguousarray(xl8.T)

    swgu, swgl = _quant_gu(_decode(sgu_p, sgu_s), 2.0)
    swd, swdl = _quant_d(_decode(sd_p, sd_s))
    xsh_full = _pairs(xh8T, 8)[0]
    xsl_full = _pairs(xl8T, 8)[0]

    in_maps = []
    for e in range(E):
        idx = idx_list[e][:C]
        xh_e = np.zeros((K, C), NP_F8)
        xh_e[:, :len(idx)] = xh8T[:, idx]
        xl_e = np.zeros((K, C), NP_F8)
        xl_e[:, :len(idx)] = xl8T[:, idx]
        pr_full = np.zeros(C, np.float32)
        pr_full[:len(idx)] = combine[idx, e] / 64.0
        wgu, wgl = _quant_gu(_decode(gu_p[e], gu_s[e]), 4.0)
        wd, wdl = _quant_d(_decode(d_p[e], d_s[e]))
        in_maps.append({
            "xh": _pairs(xh_e, 4),
            "xl": _pairs(xl_e, 4),
            "xsh": _pairs(np.ascontiguousarray(xh8T[:, e * TS:(e + 1) * TS]), 8)[0],
            "xsl": _pairs(np.ascontiguousarray(xl8T[:, e * TS:(e + 1) * TS]), 8)[0],
            "wgu": wgu, "wgl": wgl, "wd": wd, "wdl": wdl,
            "swgu": swgu, "swgl": swgl, "swd": swd, "swdl": swdl,
            "pr": np.ascontiguousarray(pr_full.reshape(C // 128, 128).T),
        })

    nc = _get_program()
    res = bass_utils.run_bass_kernel_spmd(nc, in_maps,
                                          core_ids=list(range(N_CORES)))

    out = np.zeros((T, K), np.float32)
    for e in range(E):
        idx = idx_list[e][:C]
        out[idx] += res.results[e]["y"][:len(idx)].astype(np.float32)
        out[e * TS:(e + 1) * TS] += res.results[e]["ysh"].astype(np.float32)

    if overflow:
        for e in range(E):
            extra = idx_list[e][C:]
            if len(extra) == 0:
                continue
            wgu = _decode(gu_p[e], gu_s[e])
            wd = _decode(d_p[e], d_s[e])
            h = x[extra] @ wgu
            g, u = h[:, :I], h[:, I:]
            a = (g / (1 + np.exp(-g))) * u
            out[extra] += (a @ wd) * combine[extra, e][:, None]
    return out


# revision 21
# speedup vs baseline: 1.2442x; 1.0069x over previous
"""Trainium2 Bass kernel for a quantized (FP4 e2m1, group-64 scales) MoE layer.

FP8 DoubleRow edition: all matmuls run as fp8e4 (IEEE e4m3, max 240)
DoubleRow matmuls (2 k-chunks per instruction). The host pre-scales and
pre-quantizes everything; the device does zero dequantization.

Numerics (validated against the reference on the fixed seed, rel ~1.3e-2):
  * gate weights: fp8(16*Wg) + fp8 residual (shipped, extra matmul pass)
  * up weights:   fp8(4*Wu) routed / fp8(2*Wu) shared (plain)
  * down weights: fp8(16*Wd) + fp8 residual
  * activations x: fp8(x) + fp8 residual (two moving passes)
  * act = silu(g)*u: computed in bf16, re-quantized to fp8 + fp8 residual
  * outputs fp16, combine probs folded into the ACT-engine copy scale.

Sharding: expert-parallel (core e owns routed expert e, capacity C=512)
plus a 256-token slice of the always-on shared expert per core. Token
gather/scatter and combine run on host.
"""

import numpy as np
import ml_dtypes

import concourse.bacc as bacc
import concourse.bass as bass
import concourse.mybir as mybir
import concourse.tile as tile
from concourse import bass_utils, library_config

F32 = mybir.dt.float32
BF16 = mybir.dt.bfloat16
F16 = mybir.dt.float16
FP8 = mybir.dt.float8e4
DR = mybir.MatmulPerfMode.DoubleRow
Copy = mybir.ActivationFunctionType.Copy
Silu = mybir.ActivationFunctionType.Silu
Mult = mybir.AluOpType.mult
Sub = mybir.AluOpType.subtract

NP_BF16 = ml_dtypes.bfloat16
NP_F8 = ml_dtypes.float8_e4m3          # IEEE e4m3: max 240, min normal 2^-7

T, K, I, E, TOPK, GS = 2048, 2048, 1024, 8, 2, 64
N_CORES = 8
C = 512            # routed token capacity per expert
TS = T // N_CORES  # shared-expert tokens per core = 256
KP = K // 256      # 8 contraction chunk-pairs for gate_up
IP = I // 256      # 4 contraction chunk-pairs for down

FP4_T = np.array([0, .5, 1, 1.5, 2, 3, 4, 6,
                  0, -.5, -1, -1.5, -2, -3, -4, -6], dtype=np.float32)

_COMPILED = {}


# ---------------------------------------------------------------- host prep
def _decode(packed, scales):
    """[R, N] int32 + [R*8//GS, N] scales -> [R*8, N] f32 true weights."""
    shifts = (np.arange(8, dtype=np.int32)[None, :, None] * 4)
    nib = (packed[:, None, :] >> shifts) & 0xF
    w = FP4_T[nib].reshape(packed.shape[0] * 8, packed.shape[1])
    return w * np.repeat(scales.astype(np.float32), GS, axis=0)


def _pairs(mat, block):
    """[R, N] -> [R//(256*block), 128, block*2N]: chunk pairs, `block` pairs
    side by side per DMA-able row block."""
    R, N = mat.shape
    p = mat.reshape(R // 256, 2, 128, N).transpose(0, 2, 1, 3)
    p = p.reshape(R // 256, 128, 2 * N)
    g = p.reshape(R // 256 // block, block, 128, 2 * N).transpose(0, 2, 1, 3)
    return np.ascontiguousarray(g.reshape(R // 256 // block, 128, block * 2 * N))


def _f8(a):
    return np.asarray(a, np.float32).astype(NP_F8)


def _quant_gu(wtrue, up_scale):
    """-> (w8 wave-blocks [4,128,8192], wl_gate wave-blocks [4,128,4096]).

    Wave w (output chunks 2w, 2w+1) owns gate cols [256w:256w+256) and up
    cols [I+256w:...). Each wave block packs those 512 columns for all 16
    contraction chunks so a wave's weights arrive in one ~1MB stream."""
    wg = 16.0 * wtrue[:, :I]
    wu = up_scale * wtrue[:, I:]
    w8 = _f8(np.concatenate([wg, wu], axis=1))
    wl = _f8(wg - w8[:, :I].astype(np.float32))
    wgu_w = np.stack([_pairs(np.concatenate(
        [w8[:, 256 * w:256 * w + 256], w8[:, I + 256 * w:I + 256 * w + 256]],
        axis=1), 8)[0] for w in range(4)])
    wgl_w = np.stack([_pairs(wl[:, 256 * w:256 * w + 256], 8)[0]
                      for w in range(4)])
    return wgu_w, wgl_w


def _quant_d(wtrue):
    w16 = 16.0 * wtrue
    w8 = _f8(w16)
    wl = _f8(w16 - w8.astype(np.float32))
    return _pairs(w8, 2), _pairs(wl, 2)


# ---------------------------------------------------------------- device
def _build_program(reps=1):
    nc = bacc.Bacc("TRN2", target_bir_lowering=False, debug=False,
                   num_devices=N_CORES)

    xh_d = nc.dram_tensor("xh", [2, 128, 4096], FP8, kind="ExternalInput")
    xl_d = nc.dram_tensor("xl", [2, 128, 4096], FP8, kind="ExternalInput")
    xsh_d = nc.dram_tensor("xsh", [128, 4096], FP8, kind="ExternalInput")
    xsl_d = nc.dram_tensor("xsl", [128, 4096], FP8, kind="ExternalInput")
    wgu_d = nc.dram_tensor("wgu", [4, 128, 8192], FP8, kind="ExternalInput")
    wgl_d = nc.dram_tensor("wgl", [4, 128, 4096], FP8, kind="ExternalInput")
    wd_d = nc.dram_tensor("wd", [2, 128, 8192], FP8, kind="ExternalInput")
    wdl_d = nc.dram_tensor("wdl", [2, 128, 8192], FP8, kind="ExternalInput")
    swgu_d = nc.dram_tensor("swgu", [4, 128, 8192], FP8, kind="ExternalInput")
    swgl_d = nc.dram_tensor("swgl", [4, 128, 4096], FP8, kind="ExternalInput")
    swd_d = nc.dram_tensor("swd", [2, 128, 8192], FP8, kind="ExternalInput")
    swdl_d = nc.dram_tensor("swdl", [2, 128, 8192], FP8, kind="ExternalInput")
    pr_d = nc.dram_tensor("pr", [128, C // 128], F32, kind="ExternalInput")
    y_d = nc.dram_tensor("y", [C, K], F16, kind="ExternalOutput")
    ysh_d = nc.dram_tensor("ysh", [TS, K], F16, kind="ExternalOutput")

    def two(ap):
        return ap.rearrange("p (two n) -> p two n", two=2)

    with tile.TileContext(nc) as tc:
        with (
            tc.tile_pool(name="wgu", bufs=8) as wgu_pool,
            tc.tile_pool(name="wgl", bufs=6) as wgl_pool,
            tc.tile_pool(name="wd", bufs=3) as wd_pool,
            tc.tile_pool(name="wdl", bufs=3) as wdl_pool,
            tc.tile_pool(name="x", bufs=7) as x_pool,
            tc.tile_pool(name="a8", bufs=6) as a8_pool,
            tc.tile_pool(name="al8", bufs=6) as al8_pool,
            tc.tile_pool(name="sil", bufs=4) as sil_pool,
            tc.tile_pool(name="abf", bufs=4) as abf_pool,
            tc.tile_pool(name="ysb", bufs=18) as ysb_pool,
            tc.tile_pool(name="pr", bufs=1) as pr_pool,
            tc.tile_pool(name="ps", bufs=8, space="PSUM") as psum_pool,
        ):
            nc.gpsimd.load_library(library_config.standard)

            for _rep in range(reps):
                # ---------- input DMA stream (sync queue, priority order)
                def load(pool, dram, idx, cols, tag):
                    t = pool.tile([128, cols], FP8, tag=tag)
                    nc.sync.dma_start(t[:], dram[idx, :, :] if idx is not None
                                      else dram[:, :])
                    return t

                # first-needed pieces at pair granularity so PE starts early
                xh0 = x_pool.tile([128, 4096], FP8, tag="x", name="xh0")
                nc.sync.dma_start(xh0[:, 0:1024], xh_d[0, :, 0:1024])
                wgu0 = wgu_pool.tile([128, 8192], FP8, tag="wgu", name="wgu0")
                nc.sync.dma_start(wgu0[:, 0:4096], wgu_d[0, :, 0:4096])
                xl0 = x_pool.tile([128, 4096], FP8, tag="x", name="xl0")
                nc.sync.dma_start(xl0[:, 0:1024], xl_d[0, :, 0:1024])
                nc.sync.dma_start(xh0[:, 1024:4096], xh_d[0, :, 1024:4096])
                nc.sync.dma_start(xl0[:, 1024:4096], xl_d[0, :, 1024:4096])
                nc.sync.dma_start(wgu0[:, 4096:8192], wgu_d[0, :, 4096:8192])
                xh_t = [xh0]
                xl_t = [xl0]
                wgu_t = [wgu0]
                xh_t.append(load(x_pool, xh_d, 1, 4096, "x"))
                xl_t.append(load(x_pool, xl_d, 1, 4096, "x"))
                wgl_t = [load(wgl_pool, wgl_d, 0, 4096, "wgl")]
                for q in range(1, 4):
                    wgu_t.append(load(wgu_pool, wgu_d, q, 8192, "wgu"))
                    wgl_t.append(load(wgl_pool, wgl_d, q, 4096, "wgl"))
                wd_t = [load(wd_pool, wd_d, q, 8192, "wd") for q in range(2)]
                wdl_t = [load(wdl_pool, wdl_d, q, 8192, "wdl") for q in range(2)]
                pr_t = pr_pool.tile([128, C // 128], F32, tag="pr")
                nc.sync.dma_start(pr_t[:], pr_d[:, :])
                xsh_t = [load(x_pool, xsh_d, None, 4096, "x")]
                xsl_t = [load(x_pool, xsl_d, None, 4096, "x")]
                swgu_t, swgl_t = [], []
                for q in range(4):
                    swgu_t.append(load(wgu_pool, swgu_d, q, 8192, "wgu"))
                    swgl_t.append(load(wgl_pool, swgl_d, q, 4096, "wgl"))
                swd_t = [load(wd_pool, swd_d, q, 8192, "wd") for q in range(2)]
                swdl_t = [load(wdl_pool, swdl_d, q, 8192, "wdl")
                          for q in range(2)]

                # AP helpers ------------------------------------------------
                def wgu_ap(tiles, w, j, i0):
                    # wave w, pair j of gate_up weights, 128-col window at i0
                    # (0/128 = gate c, 256/384 = up c within the wave block)
                    return two(tiles[w][:, j * 1024:(j + 1) * 1024])[
                        :, :, i0:i0 + 128]

                def wgl_ap(tiles, w, j, cw):
                    return two(tiles[w][:, j * 512:(j + 1) * 512])[
                        :, :, cw * 128:(cw + 1) * 128]

                def x_ap(tiles, j, tcnt):
                    if tcnt == C:
                        t = tiles[j // 4]
                        return two(t[:, (j % 4) * 1024:(j % 4 + 1) * 1024])
                    return two(tiles[0][:, j * 512:(j + 1) * 512])

                def wd_ap(tiles, p, w0, w1):
                    t = tiles[p // 2]
                    return two(t[:, (p % 2) * 4096:(p % 2 + 1) * 4096])[:, :, w0:w1]

                def mlp(wgu_tl, wgl_tl, wd_tl, wdl_tl, xh_tl, xl_tl, tcnt,
                        y_dram, pr_ap, actl):
                    """One expert pass: gate_up -> act -> down -> out."""
                    a8_tiles, al8_tiles = [], []
                    # ---- gate_up in waves of 2 chunks (4 psums) so PSUM
                    # banks hand off smoothly at phase boundaries ----
                    for wave in range(4):
                        cs = range(2 * wave, 2 * wave + 2)
                        gps = {c: psum_pool.tile([128, tcnt], F32, tag="ps",
                                                 name=f"gps{c}")
                               for c in cs}
                        ups = {c: psum_pool.tile([128, tcnt], F32, tag="ps",
                                                 name=f"ups{c}")
                               for c in cs}
                        for j in range(KP):
                            for c in cs:
                                nc.tensor.matmul(
                                    gps[c][:],
                                    wgu_ap(wgu_tl, wave, j, (c % 2) * 128),
                                    x_ap(xh_tl, j, tcnt),
                                    start=(j == 0), stop=False, perf_mode=DR)
                            for c in cs:
                                nc.tensor.matmul(
                                    ups[c][:],
                                    wgu_ap(wgu_tl, wave, j,
                                           256 + (c % 2) * 128),
                                    x_ap(xh_tl, j, tcnt),
                                    start=(j == 0), stop=False, perf_mode=DR)
                            for c in cs:
                                nc.tensor.matmul(
                                    gps[c][:],
                                    wgu_ap(wgu_tl, wave, j, (c % 2) * 128),
                                    x_ap(xl_tl, j, tcnt),
                                    start=False, stop=False, perf_mode=DR)
                            for c in cs:
                                nc.tensor.matmul(
                                    ups[c][:],
                                    wgu_ap(wgu_tl, wave, j,
                                           256 + (c % 2) * 128),
                                    x_ap(xl_tl, j, tcnt),
                                    start=False, stop=(j == KP - 1),
                                    perf_mode=DR)
                        # gate residual pass
                        for j in range(KP):
                            for c in cs:
                                nc.tensor.matmul(
                                    gps[c][:],
                                    wgl_ap(wgl_tl, wave, j, c % 2),
                                    x_ap(xh_tl, j, tcnt),
                                    start=False, stop=(j == KP - 1),
                                    perf_mode=DR)
                        # act: sil = silu(g), a_bf = sil*up, a8 (+ al8) fp8
                        for c in cs:
                            cp, half = c // 2, c % 2
                            if half == 0:
                                a8_tiles.append(
                                    a8_pool.tile([128, 2 * tcnt], FP8, tag="a8",
                                                 name=f"a8_{cp}"))
                                if actl:
                                    al8_tiles.append(
                                        al8_pool.tile([128, 2 * tcnt], FP8,
                                                      tag="al8",
                                                      name=f"al8_{cp}"))
                            sil = sil_pool.tile([128, tcnt], BF16, tag="sil")
                            abf = abf_pool.tile([128, tcnt], BF16, tag="abf")
                            a8s = a8_tiles[cp][:, half * tcnt:(half + 1) * tcnt]
                            # token-split act chain: the down phase's first
                            # t-block can start while the second half drains
                            ht = tcnt // 2
                            for u in range(2):
                                us = slice(u * ht, (u + 1) * ht)
                                nc.scalar.activation(sil[:, us], gps[c][:, us],
                                                     Silu, scale=1.0 / 16)
                                nc.vector.tensor_tensor(abf[:, us], sil[:, us],
                                                        ups[c][:, us], Mult)
                                nc.gpsimd.tensor_copy(a8s[:, us], abf[:, us])
                                if actl:
                                    nc.vector.tensor_tensor(
                                        al8_tiles[cp][:, half * tcnt + u * ht:
                                                       half * tcnt +
                                                       (u + 1) * ht],
                                        abf[:, us], a8s[:, us], Sub)

                    # ---- down ----
                    tblocks = tcnt // 128
                    for tb in range(tblocks):
                        for kw in range(4):
                            ps = psum_pool.tile([128, 512], F32, tag="ps")
                            for cp in range(IP):
                                nc.tensor.matmul(
                                    ps[:],
                                    two(a8_tiles[cp][:])[:, :,
                                                         tb * 128:(tb + 1) * 128],
                                    wd_ap(wd_tl, cp, kw * 512, (kw + 1) * 512),
                                    start=(cp == 0), stop=False, perf_mode=DR)
                            if actl:
                                for cp in range(IP):
                                    nc.tensor.matmul(
                                        ps[:],
                                        two(al8_tiles[cp][:])[
                                            :, :, tb * 128:(tb + 1) * 128],
                                        wd_ap(wd_tl, cp, kw * 512,
                                              (kw + 1) * 512),
                                        start=False, stop=False, perf_mode=DR)
                            for cp in range(IP):
                                nc.tensor.matmul(
                                    ps[:],
                                    two(a8_tiles[cp][:])[:, :,
                                                         tb * 128:(tb + 1) * 128],
                                    wd_ap(wdl_tl, cp, kw * 512, (kw + 1) * 512),
                                    start=False, stop=(cp == IP - 1),
                                    perf_mode=DR)
                            ot = ysb_pool.tile([128, 512], F16, tag="ysb")
                            yslice = y_dram[tb * 128:(tb + 1) * 128,
                                            kw * 512:(kw + 1) * 512]
                            scale = (pr_ap[:, tb:tb + 1] if pr_ap is not None
                                     else 1.0 / 32)
                            last = tb == tblocks - 1 and kw == 3
                            if last and pr_ap is None:
                                # final tile: halve across ACT/DVE + two DMA
                                # queues to shorten the kernel tail
                                nc.scalar.activation(ot[:, 0:256],
                                                     ps[:, 0:256], Copy,
                                                     scale=scale)
                                nc.vector.tensor_scalar_mul(ot[:, 256:512],
                                                            ps[:, 256:512],
                                                            scale)
                                nc.scalar.dma_start(yslice[:, 0:256],
                                                    ot[:, 0:256])
                                nc.sync.dma_start(yslice[:, 256:512],
                                                  ot[:, 256:512])
                            else:
                                # alternate copy engines so PSUM banks free
                                # fast; all out-DMAs ride the SP queue (inputs
                                # are long since issued; ACT queue must stay
                                # clear of transfer-holding DMACopies)
                                if (tb * 4 + kw) % 2 == 0:
                                    nc.vector.tensor_scalar_mul(ot[:], ps[:],
                                                                scale)
                                else:
                                    nc.scalar.activation(ot[:], ps[:], Copy,
                                                         scale=scale)
                                nc.sync.dma_start(yslice, ot[:])

                mlp(wgu_t, wgl_t, wd_t, wdl_t, xh_t, xl_t, C, y_d, pr_t,
                    actl=False)
                mlp(swgu_t, swgl_t, swd_t, swdl_t, xsh_t, xsl_t, TS, ysh_d,
                    None, actl=True)

    nc.compile()
    return nc


def _get_program():
    if "nc" not in _COMPILED:
        _COMPILED["nc"] = _build_program()
    return _COMPILED["nc"]


# ---------------------------------------------------------------- entry
def kernel(**inputs) -> np.ndarray:
    x = np.asarray(inputs["hidden_states"], np.float32)
    gu_p = np.asarray(inputs["gate_up_weight_packed"])
    gu_s = np.asarray(inputs["gate_up_scales"], np.float32)
    d_p = np.asarray(inputs["down_weight_packed"])
    d_s = np.asarray(inputs["down_scales"], np.float32)
    sgu_p = np.asarray(inputs["shared_gate_up_packed"])
    sgu_s = np.asarray(inputs["shared_gate_up_scales"], np.float32)
    sd_p = np.asarray(inputs["shared_down_packed"])
    sd_s = np.asarray(inputs["shared_down_scales"], np.float32)
    eids = np.asarray(inputs["expert_ids"])
    eprobs = np.asarray(inputs["expert_probs"], np.float32)

    # host routing
    combine = np.zeros((T, E), np.float32)
    np.add.at(combine, (np.arange(T)[:, None], eids), eprobs)
    idx_list = [np.nonzero(combine[:, e])[0] for e in range(E)]
    overflow = max(len(i) for i in idx_list) > C

    # x quantization (hi + residual), transposed [K, T]
    xh8 = x.astype(NP_F8)
    xl8 = (x - xh8.astype(np.float32)).astype(NP_F8)
    xh8T = np.ascontiguousarray(xh8.T)
    xl8T = np.ascontiguousarray(xl8.T)

    swgu, swgl = _quant_gu(_decode(sgu_p, sgu_s), 2.0)
    swd, swdl = _quant_d(_decode(sd_p, sd_s))
    xsh_full = _pairs(xh8T, 8)[0]
    xsl_full = _pairs(xl8T, 8)[0]

    in_maps = []
    for e in range(E):
        idx = idx_list[e][:C]
        xh_e = np.zeros((K, C), NP_F8)
        xh_e[:, :len(idx)] = xh8T[:, idx]
        xl_e = np.zeros((K, C), NP_F8)
        xl_e[:, :len(idx)] = xl8T[:, idx]
        pr_full = np.zeros(C, np.float32)
        pr_full[:len(idx)] = combine[idx, e] / 64.0
        wgu, wgl = _quant_gu(_decode(gu_p[e], gu_s[e]), 4.0)
        wd, wdl = _quant_d(_decode(d_p[e], d_s[e]))
        in_maps.append({
            "xh": _pairs(xh_e, 4),
            "xl": _pairs(xl_e, 4),
            "xsh": _pairs(np.ascontiguousarray(xh8T[:, e * TS:(e + 1) * TS]), 8)[0],
            "xsl": _pairs(np.ascontiguousarray(xl8T[:, e * TS:(e + 1) * TS]), 8)[0],
            "wgu": wgu, "wgl": wgl, "wd": wd, "wdl": wdl,
            "swgu": swgu, "swgl": swgl, "swd": swd, "swdl": swdl,
            "pr": np.ascontiguousarray(pr_full.reshape(C // 128, 128).T),
        })

    nc = _get_program()
    res = bass_utils.run_bass_kernel_spmd(nc, in_maps,
                                          core_ids=list(range(N_CORES)))

    out = np.zeros((T, K), np.float32)
    for e in range(E):
        idx = idx_list[e][:C]
        out[idx] += res.results[e]["y"][:len(idx)].astype(np.float32)
        out[e * TS:(e + 1) * TS] += res.results[e]["ysh"].astype(np.float32)

    if overflow:
        for e in range(E):
            extra = idx_list[e][C:]
            if len(extra) == 0:
                continue
            wgu = _decode(gu_p[e], gu_s[e])
            wd = _decode(d_p[e], d_s[e])
            h = x[extra] @ wgu
            g, u = h[:, :I], h[:, I:]
            a = (g / (1 + np.exp(-g))) * u
            out[extra] += (a @ wd) * combine[extra, e][:, None]
    return out
